# revision 42
# baseline (speedup 1.0000x reference)
"""XNOR-Net BasicBlock forward (BN-sign-binconv-PReLU x2 + BN + residual + PReLU)
distributed over 8 Trainium2 NeuronCores, data-parallel over the batch axis.

Self-contained: hardcodes shapes N=64, C=128, H=W=56, 8 cores.

Design notes:
- Phase A: x streams from HBM (f32) feeding BN1 bn_stats (full f32 precision:
  the sign1 threshold cascades through both binary convs, so stats from a
  rounded copy flip near-threshold pixels) and an f16 copy (xb) kept in SBUF
  for the phase-D residual. Phase B re-loads x (f32) for the sign itself.
- 3x3 binary conv: 6 fp8 DoubleRow matmul passes per 448-col PSUM tile — 3
  vertical tap pairs (dh 0+1) plus 3 (dh=2 tap, zero-weight slot) pairs. All
  rhs pair strides = PITCH = 64 (DoubleRow requires 16-aligned pair strides
  on HW; unaligned strides wedge the PE). 3 PE cycles/pixel.
- PSUM evacuation is a fused ACT Prelu with scale = s_hat (the power-of-2
  rounding of the conv scale s = mean|w|, so p' = s_hat*prelu(c) is exact in
  f16). BN's var+EPS is NOT scale-invariant (s ~ 5e-4 makes var comparable to
  EPS), so stats and thresholds are corrected by rho = s/s_hat per channel.
- sign ops: ACT Sign(scale, bias) for most images; a 2-op DVE threshold
  (is_ge + affine to fp8) for some, to balance engine load.
- BN batch stats: bn_stats/bn_aggr per core, AllGather of [C,2] partial
  moments, single strided gather DMA, on-chip reduce, Newton-refined rsqrt.
"""

import numpy as np
import ml_dtypes

import concourse.bass as bass
import concourse.mybir as mybir
import concourse.tile as tile
from concourse import bacc
from concourse.bass_utils import run_bass_kernel_spmd

F32 = mybir.dt.float32
F16 = mybir.dt.float16
FP8 = mybir.dt.float8e4
PITCH = 64
AF = mybir.ActivationFunctionType
OP = mybir.AluOpType

N_CORES = 8
N_LOC = 8          # images per core
C = 128            # channels (== partitions)
H = W = 56
HW = H * W         # 3136
EPS = 1e-5
PROWS = H + 4      # padded rows + 2 spare (garbage-col / zero-pair overreads)
TILE_ROWS = 7
N_TILES = H // TILE_ROWS     # 8
QSPAN = TILE_ROWS * PITCH    # 448 padded cols per tile
GTILES = 4                   # tiles per PSUM group (4 banks)
N_GROUPS = N_TILES // GTILES # 2
GCOMPACT = GTILES * TILE_ROWS * W   # 1568 compact elems per group
SCHUNK = TILE_ROWS * W       # 392, bn_stats chunk (4 per group)

# flat padded offsets of the 9 taps (dh*PITCH + dw), natural order; pairs are
# (0,1) (2,3) (4,5) (6,7), single tap 8 is paired with zero weights (slot 9)
TAP_OFF = [0, 1, 2, PITCH, PITCH + 1, PITCH + 2, 2 * PITCH, 2 * PITCH + 1,
           2 * PITCH + 2]
PAIR_D = [TAP_OFF[1] - TAP_OFF[0], TAP_OFF[3] - TAP_OFF[2],
          TAP_OFF[5] - TAP_OFF[4], TAP_OFF[7] - TAP_OFF[6], 1]

# pp param columns. S1H/S2H: power-of-2 rounding of the conv scales s=mean|w|
# (exact in f16); R1/R2: residual ratio rho = s/s_hat, applied to stats and
# sign thresholds so BN's var+EPS sees the reference-scaled values.
(P_G1, P_B1, P_G2, P_B2, P_G3, P_B3, P_A1, P_A2, P_A3,
 P_S1H, P_R1, P_S2H, P_R2) = range(13)
NP = 13

DR = mybir.MatmulPerfMode.DoubleRow


def _col(t, j):
    return t[:, j : j + 1]


def _rstd_from_allreduced(nc, pool, ar, name):
    """ar: [128,2] = sum over cores of [mean_i, var_i + mean_i^2].
    Returns (mean, rstd) tiles [128,1] f32 with rstd = 1/sqrt(var+EPS),
    Newton-refined to cover ScalarE Sqrt spline error."""
    mean = pool.tile([C, 1], F32, name=f"mean_{name}", tag=f"mean_{name}")
    ex2 = pool.tile([C, 1], F32, name=f"ex2_{name}", tag="sc_ex2")
    nc.vector.tensor_scalar_mul(mean[:], _col(ar, 0), 1.0 / N_CORES)
    nc.vector.tensor_scalar_mul(ex2[:], _col(ar, 1), 1.0 / N_CORES)
    negmean = pool.tile([C, 1], F32, name=f"negmean_{name}", tag="sc_negmean")
    nc.vector.tensor_scalar_mul(negmean[:], mean[:], -1.0)
    vpe = pool.tile([C, 1], F32, name=f"vpe_{name}", tag="sc_vpe")
    nc.vector.scalar_tensor_tensor(vpe[:], mean[:], negmean[:], ex2[:], OP.mult, OP.add)
    nc.vector.tensor_scalar_add(vpe[:], vpe[:], EPS)
    rec = pool.tile([C, 1], F32, name=f"rec_{name}", tag="sc_rec")
    nc.vector.reciprocal(rec[:], vpe[:])
    rstd = pool.tile([C, 1], F32, name=f"rstd_{name}", tag=f"rstd_{name}")
    nc.scalar.activation(rstd[:], rec[:], AF.Sqrt)
    # Newton: y <- y * (1.5 - 0.5 * vpe * y^2)
    t1 = pool.tile([C, 1], F32, name=f"t1_{name}", tag="sc_t1")
    nc.vector.tensor_tensor(out=t1[:], in0=rstd[:], in1=rstd[:], op=OP.mult)
    nc.vector.tensor_tensor(out=t1[:], in0=t1[:], in1=vpe[:], op=OP.mult)
    nc.vector.tensor_scalar(t1[:], t1[:], -0.5, 1.5, OP.mult, OP.add)
    nc.vector.tensor_tensor(out=rstd[:], in0=rstd[:], in1=t1[:], op=OP.mult)
    return mean, rstd


def _affine_consts(nc, pool, pp, mean, rstd, g_col, b_col, name, rho=None):
    """k = g * rstd (in reference units); cb = b - mean * k.
    If rho is given, the consumer reads the s_hat-scaled tensor, so the
    returned slope is ks = k * rho. tau = -cb/ks is the threshold in
    consumer-input units (valid for ks > 0)."""
    k = pool.tile([C, 1], F32, name=f"k_{name}", tag=f"k_{name}")
    nc.vector.tensor_tensor(out=k[:], in0=_col(pp, g_col), in1=rstd[:], op=OP.mult)
    if rho is not None:
        nc.vector.tensor_tensor(out=k[:], in0=k[:], in1=rho, op=OP.mult)
        # cb must use the reference-unit slope: cb = b - mean_ref * k_ref,
        # and mean passed in is already reference-unit, so recompute k_ref
        # separately? No: mean_ref * k_ref == (mean_ref) * (ks / rho).
        # Simpler: cb = b - (mean_ref / rho) * ks. Precompute mratio.
    negk = pool.tile([C, 1], F32, name=f"negk_{name}", tag="sc_negk")
    nc.vector.tensor_scalar_mul(negk[:], k[:], -1.0)
    cb = pool.tile([C, 1], F32, name=f"cb_{name}", tag=f"cb_{name}")
    if rho is None:
        nc.vector.scalar_tensor_tensor(
            cb[:], mean[:], negk[:], _col(pp, b_col), OP.mult, OP.add
        )
    else:
        # mean here is reference-unit; consumer-unit mean is mean/rho
        mc = pool.tile([C, 1], F32, name=f"mc_{name}", tag="sc_mc")
        rrho = pool.tile([C, 1], F32, name=f"rrho_{name}", tag="sc_rrho")
        nc.vector.reciprocal(rrho[:], rho)
        nc.vector.tensor_tensor(out=mc[:], in0=mean[:], in1=rrho[:], op=OP.mult)
        nc.vector.scalar_tensor_tensor(
            cb[:], mc[:], negk[:], _col(pp, b_col), OP.mult, OP.add
        )
    # tau = -cb/ks (threshold in consumer-input units)
    rk = pool.tile([C, 1], F32, name=f"rk_{name}", tag="sc_rk")
    nc.vector.reciprocal(rk[:], k[:])
    tau = pool.tile([C, 1], F32, name=f"tau_{name}", tag=f"tau_{name}")
    nc.vector.tensor_tensor(out=tau[:], in0=cb[:], in1=rk[:], op=OP.mult)
    nc.vector.tensor_scalar_mul(tau[:], tau[:], -1.0)
    return k, cb, tau


import os

CONV_MODE = os.environ.get("K_CONV_MODE", "dr6z")  # dr6z | dr3
GATHER1 = os.environ.get("K_GATHER1", "1") == "1"  # single gather DMA
DVE_SIGN = os.environ.get("K_DVE_SIGN", "1") == "1"


def build_nc(reps=1, dbg=False):
    nc = bacc.Bacc(None, target_bir_lowering=False, debug=False, num_devices=N_CORES)

    x_d = nc.dram_tensor("x", [N_LOC, C, HW], F32, kind="ExternalInput")
    w1_d = nc.dram_tensor("w1t", [C, 10, C], FP8, kind="ExternalInput")
    w2_d = nc.dram_tensor("w2t", [C, 10, C], FP8, kind="ExternalInput")
    pp_d = nc.dram_tensor("pp", [C, NP], F32, kind="ExternalInput")
    out_d = nc.dram_tensor("out", [N_LOC, C, HW], F16, kind="ExternalOutput")
    if dbg:
        dbg_pad_d = nc.dram_tensor("dbg_pad", [C, PROWS, PITCH], FP8,
                                   kind="ExternalOutput")
        dbg_c1_d = nc.dram_tensor("dbg_c1", [C, N_LOC, HW], F16,
                                  kind="ExternalOutput")
        dbg_c2_d = nc.dram_tensor("dbg_c2", [C, N_LOC, HW], F16,
                                  kind="ExternalOutput")
        dbg_k_d = nc.dram_tensor("dbg_k", [C, 8], F32, kind="ExternalOutput")

    with tile.TileContext(nc) as tc:
        with (
            tc.tile_pool(name="const", bufs=1) as const,
            tc.tile_pool(name="work", bufs=2) as work,
            tc.tile_pool(name="psum", bufs=2, space="PSUM") as psum,
            tc.tile_pool(name="dram", bufs=1, space="DRAM") as dram,
        ):
            # ---- persistent SBUF tensors ----
            pp = const.tile([C, NP], F32)
            nc.gpsimd.dma_start(pp[:], pp_d[:])
            w1s = const.tile([C, 10, C], FP8)
            w2s = const.tile([C, 10, C], FP8)
            nc.gpsimd.dma_start(w1s[:], w1_d[:])
            nc.gpsimd.dma_start(w2s[:], w2_d[:])
            xb = const.tile([C, N_LOC, HW], F16)    # f16 copy of x
            c1f = const.tile([C, N_LOC, HW], F16)   # p1 = prelu_a1(c1)
            c2f = const.tile([C, N_LOC, HW], F16)   # p2 = prelu_a2(c2)
            stats1 = const.tile([C, N_LOC * 8, 6], F32, tag="stats1")
            stats2 = const.tile([C, N_LOC * 8, 6], F32, tag="stats2")
            stats3 = const.tile([C, N_LOC * 8, 6], F32, tag="stats3")
            N_PADS = 3
            pads = []
            for j in range(N_PADS):
                p = const.tile([C, PROWS, PITCH], FP8, name=f"pad{j}")
                nc.gpsimd.memset(p[:], 0.0)
                pads.append(p)

            a1 = _col(pp, P_A1)
            a2 = _col(pp, P_A2)
            a3 = _col(pp, P_A3)
            s1h = _col(pp, P_S1H)
            s2h = _col(pp, P_S2H)
            r1 = _col(pp, P_R1)
            r2 = _col(pp, P_R2)
            r1sq = const.tile([C, 1], F32, name="r1sq")
            nc.vector.tensor_tensor(out=r1sq[:], in0=r1, in1=r1, op=OP.mult)
            r2sq = const.tile([C, 1], F32, name="r2sq")
            nc.vector.tensor_tensor(out=r2sq[:], in0=r2, in1=r2, op=OP.mult)

            cc_counter = [0]

            def reduce_stats(stats, idx, rho=None, rhosq=None):
                """bn_aggr + pack [mean, var+mean^2] (rescaled into reference
                units by rho) + allgather-sum. Returns [128,2] tile of global
                [sum mean_i, sum (var_i+m_i^2)]."""
                mv = const.tile([C, 2], F32, name=f"mv{idx}", tag="sc_mv")
                nc.vector.bn_aggr(mv[:], stats[:])
                e = const.tile([C, 2], F32, name=f"e{idx}", tag="sc_e")
                if rho is None:
                    nc.vector.tensor_copy(_col(e, 0), _col(mv, 0))
                else:
                    nc.vector.tensor_tensor(out=_col(e, 0), in0=_col(mv, 0),
                                            in1=rho, op=OP.mult)
                nc.vector.scalar_tensor_tensor(
                    _col(e, 1), _col(mv, 0), _col(mv, 0), _col(mv, 1), OP.mult, OP.add
                )
                if rhosq is not None:
                    nc.vector.tensor_tensor(out=_col(e, 1), in0=_col(e, 1),
                                            in1=rhosq, op=OP.mult)
                n = cc_counter[0]
                cc_counter[0] += 1
                cci = dram.tile([C, 2], F32, name=f"cc_in{n}", tag=f"cc_in{n}")
                cco = dram.tile([N_CORES, C, 2], F32, name=f"cc_out{n}",
                                tag=f"cc_out{n}", addr_space="Shared")
                nc.sync.dma_start(cci[:], e[:])
                nc.gpsimd.collective_compute(
                    "AllGather",
                    OP.bypass,
                    replica_groups=[list(range(N_CORES))],
                    ins=[cci.opt()],
                    outs=[cco.opt()],
                )
                # gather: g8[c, j, r] <- cco[r, c, j]
                g8 = const.tile([C, 2, N_CORES], F32, name=f"g8{idx}", tag="sc_g8")
                if GATHER1:
                    c0 = cco[0]
                    in_ap = bass.AP(c0.tensor, c0.offset,
                                    [list(c0.ap[0]), [1, 2], [2 * C, N_CORES]])
                    nc.sync.dma_start(g8[:], in_ap)
                else:
                    for r in range(N_CORES):
                        nc.sync.dma_start(g8[:, :, r], cco[r])
                g = const.tile([C, 2], F32, name=f"g{idx}", tag="sc_g")
                nc.vector.tensor_reduce(g[:], g8[:], mybir.AxisListType.X, OP.add)
                return g

            def conv(pad, ws, dst, stats, i, acol, shcol):
                """3x3 conv of padded +/-1 fp8 image (row pitch 64) -> PReLU'd
                f16 dst [C,HW]. 5 fp8 DoubleRow passes per 448-col tile (pass 4
                pairs the last tap with zero weights). Evacuation fuses PReLU;
                engine alternates ACT/DVE for load balance; bn_stats chunks
                follow each group."""
                padf = pad[:].rearrange("p r w -> p (r w)")
                for g in range(N_GROUPS):
                    tiles = range(g * GTILES, (g + 1) * GTILES)
                    psg = psum.tile([C, GTILES, 512], F32, tag="ps",
                                    name=f"ps{g}", bufs=2)
                    if CONV_MODE == "dr6z":
                        # 6 DoubleRow passes, all rhs pair strides = PITCH
                        # (16B-aligned, HW requirement): 3 vertical tap pairs
                        # (dh 0+1) + 3 (dh=2 tap, zero-slot-9) pairs.
                        wb = ws[:, 0, :]
                        for p_ in range(6):
                            if p_ < 3:
                                woff, wstride, base = p_ * C, 3 * C, TAP_OFF[p_]
                            else:
                                woff = (3 + p_) * C      # slots 6,7,8
                                wstride = (9 - (3 + p_)) * C  # to zero slot 9
                                base = TAP_OFF[3 + p_]
                            wp = bass.AP(wb.tensor, wb.offset + woff,
                                         [list(wb.ap[0]), [wstride, 2], [1, C]])
                            for j, t in enumerate(tiles):
                                q0 = t * QSPAN + base
                                rhs = bass.AP(padf.tensor, padf.offset + q0,
                                              [list(padf.ap[0]), [PITCH, 2],
                                               [1, QSPAN]])
                                nc.tensor.matmul(
                                    psg[:, j, 0:QSPAN], wp, rhs,
                                    start=(p_ == 0), stop=(p_ == 5),
                                    perf_mode=DR,
                                )
                    else:
                        # dr3: vertical pairs (0,dw)+(1,dw) with rhs pair
                        # stride PITCH, then 3 plain passes for dh=2 taps.
                        wb = ws[:, 0, :]
                        for dw in range(3):
                            wp = bass.AP(wb.tensor, wb.offset + dw * C,
                                         [list(wb.ap[0]), [3 * C, 2], [1, C]])
                            for j, t in enumerate(tiles):
                                q0 = t * QSPAN + dw
                                rhs = bass.AP(padf.tensor, padf.offset + q0,
                                              [list(padf.ap[0]), [PITCH, 2],
                                               [1, QSPAN]])
                                nc.tensor.matmul(
                                    psg[:, j, 0:QSPAN], wp, rhs,
                                    start=(dw == 0), stop=False,
                                    perf_mode=DR,
                                )
                        for dw in range(3):
                            for j, t in enumerate(tiles):
                                q0 = t * QSPAN + 2 * PITCH + dw
                                nc.tensor.matmul(
                                    psg[:, j, 0:QSPAN], ws[:, 6 + dw, :],
                                    padf[:, q0 : q0 + QSPAN],
                                    start=False, stop=(dw == 2),
                                )
                    gbase = psg[:]
                    # For the batch's last image, split the final group's
                    # evacuation in half so its bn_stats (which gate the
                    # collective) start one half earlier.
                    nsplit = 2 if (i == N_LOC - 1 and g == N_GROUPS - 1) else 1
                    tper = GTILES // nsplit
                    for h in range(nsplit):
                        src_ap = bass.AP(gbase.tensor,
                                         gbase.offset + h * tper * 512,
                                         [list(gbase.ap[0]), [512, tper],
                                          [PITCH, TILE_ROWS], [1, W]])
                        off = g * GCOMPACT + h * tper * SCHUNK
                        dst_sl = dst[:, off : off + tper * SCHUNK]
                        dst_ap = dst_sl.rearrange("p (t r w) -> p t r w",
                                                  t=tper, r=TILE_ROWS, w=W)
                        nc.scalar.activation(dst_ap, src_ap, AF.Prelu,
                                             alpha=acol, scale=shcol)
                        for k in range(tper):
                            kk = h * tper + k
                            nc.vector.bn_stats(
                                stats[:, i * 8 + g * GTILES + kk, :],
                                dst[:, g * GCOMPACT + kk * SCHUNK
                                       : g * GCOMPACT + (kk + 1) * SCHUNK])

            for _rep in range(reps):
                # ============ Phase A: load x, convert f16, BN1 stats ========
                # Last image loads in quarters so its final bn_stats (which
                # gate the AR1 collective) start a quarter-chunk earlier.
                for i in range(N_LOC):
                    # last image: DMA in quarters (into the same half-tile) so
                    # its final bn_stats, which gate AR1, start earlier
                    ndma = 1 if i < N_LOC - 1 else 2
                    for h in range(2):
                        xin = work.tile([C, HW // 2], F32, tag="xin", bufs=3,
                                        name=f"xa{i}_{h}")
                        qsz = (HW // 2) // ndma
                        for q in range(ndma):
                            nc.sync.dma_start(
                                xin[:, q * qsz : (q + 1) * qsz],
                                x_d[i, :, h * (HW // 2) + q * qsz
                                      : h * (HW // 2) + (q + 1) * qsz])
                        xbsl = xb[:, i, h * (HW // 2) : (h + 1) * (HW // 2)]
                        nc.scalar.activation(xbsl, xin[:], AF.Copy)
                        for k in range(4):
                            # stats from the f32 data: the f16 copy shifts the
                            # BN1 mean enough (~1e-6) to flip near-threshold
                            # signs, which cascades through both binary convs
                            nc.vector.bn_stats(
                                stats1[:, i * 8 + h * 4 + k, :],
                                xin[:, k * SCHUNK : (k + 1) * SCHUNK])

                g1ar = reduce_stats(stats1, 0)
                mean1, rstd1 = _rstd_from_allreduced(nc, const, g1ar, "1")
                k1, c1b, tau1 = _affine_consts(nc, const, pp, mean1, rstd1,
                                               P_G1, P_B1, "1")

                def sign_to_pad(i, src_img, k, cb, tau):
                    """pad interior <- sign(k*src + cb) as +/-1 fp8.
                    DVE (2-op threshold) for some images, ACT for the rest.
                    High priority: the sign gates the PE for this image."""
                    pad = pads[i % N_PADS]
                    dst = pad[:, 1 : H + 1, 1 : W + 1]
                    with tc.high_priority(offset=60):
                        if DVE_SIGN and i in (1, 4, 6):
                            t01 = work.tile([C, HW], F16, tag="d1", bufs=3,
                                            name=f"t01_{i}")
                            nc.vector.tensor_scalar(t01[:], src_img, tau[:],
                                                    None, OP.is_ge)
                            nc.vector.tensor_scalar(
                                dst,
                                t01[:].rearrange("p (h w) -> p h w", h=H, w=W),
                                2.0, -1.0, OP.mult, OP.add)
                        else:
                            nc.scalar.activation(
                                dst,
                                src_img.rearrange("p (h w) -> p h w", h=H, w=W),
                                AF.Sign, bias=cb[:], scale=k[:],
                            )
                    return pad

                # ============ Phase B: b1 = sign(BN1(x)); conv1; stats2 ======
                # sign1 thresholds x near tau1 and errors cascade through two
                # binary convs, so it must read x at full f32 precision:
                # re-load x from HBM (prefetches during phase A / AR1).
                HROWS = H // 2  # 28
                for i in range(N_LOC):
                    pad = pads[i % N_PADS]
                    for h in range(2):
                        # DMA at normal priority (must not preempt phase A's
                        # final loads on the queue); sign ops high priority.
                        xin = work.tile([C, HW // 2], F32, tag="xin",
                                        bufs=3, name=f"xs{i}_{h}")
                        nc.sync.dma_start(
                            xin[:],
                            x_d[i, :, h * (HW // 2) : (h + 1) * (HW // 2)])
                        with tc.high_priority(offset=60):
                            dsth = pad[:, 1 + h * HROWS : 1 + (h + 1) * HROWS,
                                       1 : W + 1]
                            if DVE_SIGN and i in (1, 4, 6):
                                t01 = work.tile([C, HW], F16, tag="d1",
                                                bufs=3, name=f"t01_{i}_{h}")
                                t01h = t01[:, 0 : HW // 2]
                                nc.vector.tensor_scalar(t01h, xin[:], tau1[:],
                                                        None, OP.is_ge)
                                nc.vector.tensor_scalar(
                                    dsth,
                                    t01h.rearrange("p (h w) -> p h w",
                                                   h=HROWS, w=W),
                                    2.0, -1.0, OP.mult, OP.add)
                            else:
                                nc.scalar.activation(
                                    dsth,
                                    xin[:].rearrange("p (h w) -> p h w",
                                                     h=HROWS, w=W),
                                    AF.Sign, bias=c1b[:], scale=k1[:],
                                )
                    if dbg and i == 0:
                        nc.sync.dma_start(dbg_pad_d[:], pad[:])
                    conv(pad, w1s, c1f[:, i, :], stats2, i, a1, s1h)

                g2ar = reduce_stats(stats2, 1, r1, r1sq[:])
                mean2, rstd2 = _rstd_from_allreduced(nc, const, g2ar, "2")
                k2, c2b, tau2 = _affine_consts(nc, const, pp, mean2, rstd2,
                                               P_G2, P_B2, "2", rho=r1)

                # ============ Phase C: b2 = sign(BN2(p1)); conv2; stats3 =====
                for i in range(N_LOC):
                    pad = sign_to_pad(i, c1f[:, i, :], k2, c2b, tau2)
                    conv(pad, w2s, c2f[:, i, :], stats3, i, a2, s2h)

                g3ar = reduce_stats(stats3, 2, r2, r2sq[:])
                mean3, rstd3 = _rstd_from_allreduced(nc, const, g3ar, "3")
                k3, c3b, _tau3 = _affine_consts(nc, const, pp, mean3, rstd3,
                                                P_G3, P_B3, "3", rho=r2)

                if dbg:
                    nc.sync.dma_start(dbg_c1_d[:], c1f[:])
                    nc.sync.dma_start(dbg_c2_d[:], c2f[:])
                    dbgk = const.tile([C, 8], F32)
                    for j, t_ in enumerate(
                        [k1, c1b, tau1, k2, c2b, tau2, k3, c3b]
                    ):
                        nc.vector.tensor_copy(_col(dbgk, j), t_[:])
                    nc.sync.dma_start(dbg_k_d[:], dbgk[:])

                # ====== Phase D: y = PReLU(k3*p2 + x + c3b) ======
                for i in range(N_LOC):
                    d1 = work.tile([C, HW], F16, tag="d1", bufs=3)
                    nc.vector.tensor_scalar(d1[:], c2f[:, i, :], k3[:],
                                            None, OP.mult)
                    nc.vector.tensor_tensor(out=d1[:], in0=d1[:],
                                            in1=xb[:, i, :], op=OP.add)
                    if i >= N_LOC - 2:
                        # halve the trailing prelu+DMA chain of the last images
                        for h in range(2):
                            sl = slice(h * (HW // 2), (h + 1) * (HW // 2))
                            nc.scalar.activation(c1f[:, i, sl], d1[:, sl],
                                                 AF.Prelu, bias=c3b[:],
                                                 alpha=a3)
                            nc.sync.dma_start(out_d[i][:, sl], c1f[:, i, sl])
                    else:
                        nc.scalar.activation(c1f[:, i, :], d1[:], AF.Prelu,
                                             bias=c3b[:], alpha=a3)
                        nc.sync.dma_start(out_d[i], c1f[:, i, :])

    nc.compile()
    return nc


def _prep_host(x, bn1_g, bn1_b, w1, prelu1_a, bn2_g, bn2_b, w2, prelu2_a,
               bn3_g, bn3_b, prelu3_a):
    def wprep(w_flat):
        w = np.asarray(w_flat, np.float32).reshape(C, C, 3, 3)
        # lhsT layout [i, slot, o] = sign(w[o, i, dh, dw]); slot 9 zero-padded
        wT = np.sign(w).transpose(1, 2, 3, 0).reshape(C, 9, C)
        w10 = np.zeros((C, 10, C), np.float32)
        w10[:, :9, :] = wT
        s = np.mean(np.abs(w), axis=(1, 2, 3)).astype(np.float32)  # [C] per o
        s_hat = np.exp2(np.round(np.log2(s))).astype(np.float32)
        rho = (s / s_hat).astype(np.float32)
        return w10.astype(mybir.dt.np(FP8)), s_hat, rho

    w1t, s1h_, r1_ = wprep(w1)
    w2t, s2h_, r2_ = wprep(w2)

    pp = np.zeros((C, NP), np.float32)
    pp[:, P_S1H] = s1h_
    pp[:, P_R1] = r1_
    pp[:, P_S2H] = s2h_
    pp[:, P_R2] = r2_
    pp[:, P_G1] = np.asarray(bn1_g, np.float32)
    pp[:, P_B1] = np.asarray(bn1_b, np.float32)
    pp[:, P_G2] = np.asarray(bn2_g, np.float32)
    pp[:, P_B2] = np.asarray(bn2_b, np.float32)
    pp[:, P_G3] = np.asarray(bn3_g, np.float32)
    pp[:, P_B3] = np.asarray(bn3_b, np.float32)
    pp[:, P_A1] = np.float32(prelu1_a)
    pp[:, P_A2] = np.float32(prelu2_a)
    pp[:, P_A3] = np.float32(prelu3_a)

    x = np.ascontiguousarray(np.asarray(x, np.float32).reshape(64, C, HW))
    in_maps = []
    for r in range(N_CORES):
        in_maps.append({
            "x": x[r * N_LOC : (r + 1) * N_LOC],
            "w1t": w1t,
            "w2t": w2t,
            "pp": pp,
        })
    return in_maps


_NC_CACHE = None


def _get_nc():
    global _NC_CACHE
    if _NC_CACHE is None:
        _NC_CACHE = build_nc()
    return _NC_CACHE


def run(in_maps, **kwargs):
    nc = _get_nc()
    return run_bass_kernel_spmd(nc, in_maps, core_ids=list(range(N_CORES)), **kwargs)


def kernel(**inputs):
    in_maps = _prep_host(**inputs)
    last_err = None
    for attempt in range(3):
        try:
            res = run(in_maps)
            break
        except Exception as e:  # transient NRT device errors happen; retry
            last_err = e
            import time as _time
            _time.sleep(2.0)
    else:
        raise last_err
    out = np.concatenate(
        [np.asarray(r["out"]).astype(np.float32).reshape(N_LOC, C, H, W)
         for r in res.results], axis=0
    )
    return out


if __name__ == "__main__":
    rng = np.random.default_rng(0)
    x = rng.standard_normal((64, C, H, W), dtype=np.float32)
    w1 = ((rng.random((C * C * 9, 1), dtype=np.float32) - 0.5) * 0.002)
    w2 = ((rng.random((C * C * 9, 1), dtype=np.float32) - 0.5) * 0.002)
    ones = np.ones(C, np.float32)
    zeros = np.zeros(C, np.float32)
    y = kernel(x=x, bn1_g=ones, bn1_b=zeros, w1=w1, prelu1_a=np.float32(0.25),
               bn2_g=ones, bn2_b=zeros, w2=w2, prelu2_a=np.float32(0.25),
               bn3_g=ones, bn3_b=zeros, prelu3_a=np.float32(0.25))
    print("out", y.shape, y.dtype, float(np.abs(y).mean()))


# revision 43
# speedup vs baseline: 1.0001x; 1.0001x over previous
"""XNOR-Net BasicBlock forward (BN-sign-binconv-PReLU x2 + BN + residual + PReLU)
distributed over 8 Trainium2 NeuronCores, data-parallel over the batch axis.

Self-contained: hardcodes shapes N=64, C=128, H=W=56, 8 cores.

Design notes:
- Phase A: x streams from HBM (f32) feeding BN1 bn_stats (full f32 precision:
  the sign1 threshold cascades through both binary convs, so stats from a
  rounded copy flip near-threshold pixels) and an f16 copy (xb) kept in SBUF
  for the phase-D residual. Phase B re-loads x (f32) for the sign itself.
- 3x3 binary conv: 6 fp8 DoubleRow matmul passes per 448-col PSUM tile — 3
  vertical tap pairs (dh 0+1) plus 3 (dh=2 tap, zero-weight slot) pairs. All
  rhs pair strides = PITCH = 64 (DoubleRow requires 16-aligned pair strides
  on HW; unaligned strides wedge the PE). 3 PE cycles/pixel.
- PSUM evacuation is a fused ACT Prelu with scale = s_hat (the power-of-2
  rounding of the conv scale s = mean|w|, so p' = s_hat*prelu(c) is exact in
  f16). BN's var+EPS is NOT scale-invariant (s ~ 5e-4 makes var comparable to
  EPS), so stats and thresholds are corrected by rho = s/s_hat per channel.
- sign ops: ACT Sign(scale, bias) for most images; a 2-op DVE threshold
  (is_ge + affine to fp8) for some, to balance engine load.
- BN batch stats: bn_stats/bn_aggr per core, AllGather of [C,2] partial
  moments, single strided gather DMA, on-chip reduce, Newton-refined rsqrt.
"""

import numpy as np
import ml_dtypes

import concourse.bass as bass
import concourse.mybir as mybir
import concourse.tile as tile
from concourse import bacc
from concourse.bass_utils import run_bass_kernel_spmd

F32 = mybir.dt.float32
F16 = mybir.dt.float16
FP8 = mybir.dt.float8e4
PITCH = 64
AF = mybir.ActivationFunctionType
OP = mybir.AluOpType

N_CORES = 8
N_LOC = 8          # images per core
C = 128            # channels (== partitions)
H = W = 56
HW = H * W         # 3136
EPS = 1e-5
PROWS = H + 4      # padded rows + 2 spare (garbage-col / zero-pair overreads)
TILE_ROWS = 7
N_TILES = H // TILE_ROWS     # 8
QSPAN = TILE_ROWS * PITCH    # 448 padded cols per tile
GTILES = 4                   # tiles per PSUM group (4 banks)
N_GROUPS = N_TILES // GTILES # 2
GCOMPACT = GTILES * TILE_ROWS * W   # 1568 compact elems per group
SCHUNK = TILE_ROWS * W       # 392, bn_stats chunk (4 per group)

# flat padded offsets of the 9 taps (dh*PITCH + dw), natural order; pairs are
# (0,1) (2,3) (4,5) (6,7), single tap 8 is paired with zero weights (slot 9)
TAP_OFF = [0, 1, 2, PITCH, PITCH + 1, PITCH + 2, 2 * PITCH, 2 * PITCH + 1,
           2 * PITCH + 2]
PAIR_D = [TAP_OFF[1] - TAP_OFF[0], TAP_OFF[3] - TAP_OFF[2],
          TAP_OFF[5] - TAP_OFF[4], TAP_OFF[7] - TAP_OFF[6], 1]

# pp param columns. S1H/S2H: power-of-2 rounding of the conv scales s=mean|w|
# (exact in f16); R1/R2: residual ratio rho = s/s_hat, applied to stats and
# sign thresholds so BN's var+EPS sees the reference-scaled values.
(P_G1, P_B1, P_G2, P_B2, P_G3, P_B3, P_A1, P_A2, P_A3,
 P_S1H, P_R1, P_S2H, P_R2) = range(13)
NP = 13

DR = mybir.MatmulPerfMode.DoubleRow


def _col(t, j):
    return t[:, j : j + 1]


def _rstd_from_allreduced(nc, pool, ar, name):
    """ar: [128,2] = sum over cores of [mean_i, var_i + mean_i^2].
    Returns (mean, rstd) tiles [128,1] f32 with rstd = 1/sqrt(var+EPS),
    Newton-refined to cover ScalarE Sqrt spline error."""
    mean = pool.tile([C, 1], F32, name=f"mean_{name}", tag=f"mean_{name}")
    ex2 = pool.tile([C, 1], F32, name=f"ex2_{name}", tag="sc_ex2")
    nc.vector.tensor_scalar_mul(mean[:], _col(ar, 0), 1.0 / N_CORES)
    nc.vector.tensor_scalar_mul(ex2[:], _col(ar, 1), 1.0 / N_CORES)
    negmean = pool.tile([C, 1], F32, name=f"negmean_{name}", tag="sc_negmean")
    nc.vector.tensor_scalar_mul(negmean[:], mean[:], -1.0)
    vpe = pool.tile([C, 1], F32, name=f"vpe_{name}", tag="sc_vpe")
    nc.vector.scalar_tensor_tensor(vpe[:], mean[:], negmean[:], ex2[:], OP.mult, OP.add)
    nc.vector.tensor_scalar_add(vpe[:], vpe[:], EPS)
    rec = pool.tile([C, 1], F32, name=f"rec_{name}", tag="sc_rec")
    nc.vector.reciprocal(rec[:], vpe[:])
    rstd = pool.tile([C, 1], F32, name=f"rstd_{name}", tag=f"rstd_{name}")
    nc.scalar.activation(rstd[:], rec[:], AF.Sqrt)
    # Newton: y <- y * (1.5 - 0.5 * vpe * y^2)
    t1 = pool.tile([C, 1], F32, name=f"t1_{name}", tag="sc_t1")
    nc.vector.tensor_tensor(out=t1[:], in0=rstd[:], in1=rstd[:], op=OP.mult)
    nc.vector.tensor_tensor(out=t1[:], in0=t1[:], in1=vpe[:], op=OP.mult)
    nc.vector.tensor_scalar(t1[:], t1[:], -0.5, 1.5, OP.mult, OP.add)
    nc.vector.tensor_tensor(out=rstd[:], in0=rstd[:], in1=t1[:], op=OP.mult)
    return mean, rstd


def _affine_consts(nc, pool, pp, mean, rstd, g_col, b_col, name, rho=None):
    """k = g * rstd (in reference units); cb = b - mean * k.
    If rho is given, the consumer reads the s_hat-scaled tensor, so the
    returned slope is ks = k * rho. tau = -cb/ks is the threshold in
    consumer-input units (valid for ks > 0)."""
    k = pool.tile([C, 1], F32, name=f"k_{name}", tag=f"k_{name}")
    nc.vector.tensor_tensor(out=k[:], in0=_col(pp, g_col), in1=rstd[:], op=OP.mult)
    if rho is not None:
        nc.vector.tensor_tensor(out=k[:], in0=k[:], in1=rho, op=OP.mult)
        # cb must use the reference-unit slope: cb = b - mean_ref * k_ref,
        # and mean passed in is already reference-unit, so recompute k_ref
        # separately? No: mean_ref * k_ref == (mean_ref) * (ks / rho).
        # Simpler: cb = b - (mean_ref / rho) * ks. Precompute mratio.
    negk = pool.tile([C, 1], F32, name=f"negk_{name}", tag="sc_negk")
    nc.vector.tensor_scalar_mul(negk[:], k[:], -1.0)
    cb = pool.tile([C, 1], F32, name=f"cb_{name}", tag=f"cb_{name}")
    if rho is None:
        nc.vector.scalar_tensor_tensor(
            cb[:], mean[:], negk[:], _col(pp, b_col), OP.mult, OP.add
        )
    else:
        # mean here is reference-unit; consumer-unit mean is mean/rho
        mc = pool.tile([C, 1], F32, name=f"mc_{name}", tag="sc_mc")
        rrho = pool.tile([C, 1], F32, name=f"rrho_{name}", tag="sc_rrho")
        nc.vector.reciprocal(rrho[:], rho)
        nc.vector.tensor_tensor(out=mc[:], in0=mean[:], in1=rrho[:], op=OP.mult)
        nc.vector.scalar_tensor_tensor(
            cb[:], mc[:], negk[:], _col(pp, b_col), OP.mult, OP.add
        )
    # tau = -cb/ks (threshold in consumer-input units)
    rk = pool.tile([C, 1], F32, name=f"rk_{name}", tag="sc_rk")
    nc.vector.reciprocal(rk[:], k[:])
    tau = pool.tile([C, 1], F32, name=f"tau_{name}", tag=f"tau_{name}")
    nc.vector.tensor_tensor(out=tau[:], in0=cb[:], in1=rk[:], op=OP.mult)
    nc.vector.tensor_scalar_mul(tau[:], tau[:], -1.0)
    return k, cb, tau


import os

CONV_MODE = os.environ.get("K_CONV_MODE", "dr6z")  # dr6z | dr3
GATHER1 = os.environ.get("K_GATHER1", "1") == "1"  # single gather DMA
DVE_SIGN = os.environ.get("K_DVE_SIGN", "1") == "1"


def build_nc(reps=1, dbg=False):
    nc = bacc.Bacc(None, target_bir_lowering=False, debug=False, num_devices=N_CORES)

    x_d = nc.dram_tensor("x", [N_LOC, C, HW], F32, kind="ExternalInput")
    w1_d = nc.dram_tensor("w1t", [C, 10, C], FP8, kind="ExternalInput")
    w2_d = nc.dram_tensor("w2t", [C, 10, C], FP8, kind="ExternalInput")
    pp_d = nc.dram_tensor("pp", [C, NP], F32, kind="ExternalInput")
    out_d = nc.dram_tensor("out", [N_LOC, C, HW], F16, kind="ExternalOutput")
    if dbg:
        dbg_pad_d = nc.dram_tensor("dbg_pad", [C, PROWS, PITCH], FP8,
                                   kind="ExternalOutput")
        dbg_c1_d = nc.dram_tensor("dbg_c1", [C, N_LOC, HW], F16,
                                  kind="ExternalOutput")
        dbg_c2_d = nc.dram_tensor("dbg_c2", [C, N_LOC, HW], F16,
                                  kind="ExternalOutput")
        dbg_k_d = nc.dram_tensor("dbg_k", [C, 8], F32, kind="ExternalOutput")

    with tile.TileContext(nc) as tc:
        with (
            tc.tile_pool(name="const", bufs=1) as const,
            tc.tile_pool(name="work", bufs=2) as work,
            tc.tile_pool(name="psum", bufs=2, space="PSUM") as psum,
            tc.tile_pool(name="dram", bufs=1, space="DRAM") as dram,
        ):
            # ---- persistent SBUF tensors ----
            pp = const.tile([C, NP], F32)
            nc.gpsimd.dma_start(pp[:], pp_d[:])
            w1s = const.tile([C, 10, C], FP8)
            w2s = const.tile([C, 10, C], FP8)
            nc.gpsimd.dma_start(w1s[:], w1_d[:])
            nc.gpsimd.dma_start(w2s[:], w2_d[:])
            xb = const.tile([C, N_LOC, HW], F16)    # f16 copy of x
            c1f = const.tile([C, N_LOC, HW], F16)   # p1 = prelu_a1(c1)
            c2f = const.tile([C, N_LOC, HW], F16)   # p2 = prelu_a2(c2)
            stats1 = const.tile([C, N_LOC * 8, 6], F32, tag="stats1")
            stats2 = const.tile([C, N_LOC * 8, 6], F32, tag="stats2")
            stats3 = const.tile([C, N_LOC * 8, 6], F32, tag="stats3")
            N_PADS = 3
            pads = []
            for j in range(N_PADS):
                p = const.tile([C, PROWS, PITCH], FP8, name=f"pad{j}")
                nc.gpsimd.memset(p[:], 0.0)
                pads.append(p)

            a1 = _col(pp, P_A1)
            a2 = _col(pp, P_A2)
            a3 = _col(pp, P_A3)
            s1h = _col(pp, P_S1H)
            s2h = _col(pp, P_S2H)
            r1 = _col(pp, P_R1)
            r2 = _col(pp, P_R2)
            r1sq = const.tile([C, 1], F32, name="r1sq")
            nc.vector.tensor_tensor(out=r1sq[:], in0=r1, in1=r1, op=OP.mult)
            r2sq = const.tile([C, 1], F32, name="r2sq")
            nc.vector.tensor_tensor(out=r2sq[:], in0=r2, in1=r2, op=OP.mult)

            cc_counter = [0]

            def reduce_stats(stats, idx, rho=None, rhosq=None):
                """bn_aggr + pack [mean, var+mean^2] (rescaled into reference
                units by rho) + allgather-sum. Returns [128,2] tile of global
                [sum mean_i, sum (var_i+m_i^2)]."""
                mv = const.tile([C, 2], F32, name=f"mv{idx}", tag="sc_mv")
                nc.vector.bn_aggr(mv[:], stats[:])
                e = const.tile([C, 2], F32, name=f"e{idx}", tag="sc_e")
                if rho is None:
                    nc.vector.tensor_copy(_col(e, 0), _col(mv, 0))
                else:
                    nc.vector.tensor_tensor(out=_col(e, 0), in0=_col(mv, 0),
                                            in1=rho, op=OP.mult)
                nc.vector.scalar_tensor_tensor(
                    _col(e, 1), _col(mv, 0), _col(mv, 0), _col(mv, 1), OP.mult, OP.add
                )
                if rhosq is not None:
                    nc.vector.tensor_tensor(out=_col(e, 1), in0=_col(e, 1),
                                            in1=rhosq, op=OP.mult)
                n = cc_counter[0]
                cc_counter[0] += 1
                cci = dram.tile([C, 2], F32, name=f"cc_in{n}", tag=f"cc_in{n}")
                cco = dram.tile([N_CORES, C, 2], F32, name=f"cc_out{n}",
                                tag=f"cc_out{n}", addr_space="Shared")
                nc.sync.dma_start(cci[:], e[:])
                nc.gpsimd.collective_compute(
                    "AllGather",
                    OP.bypass,
                    replica_groups=[list(range(N_CORES))],
                    ins=[cci.opt()],
                    outs=[cco.opt()],
                )
                # gather: g8[c, j, r] <- cco[r, c, j]
                g8 = const.tile([C, 2, N_CORES], F32, name=f"g8{idx}", tag="sc_g8")
                if GATHER1:
                    c0 = cco[0]
                    in_ap = bass.AP(c0.tensor, c0.offset,
                                    [list(c0.ap[0]), [1, 2], [2 * C, N_CORES]])
                    nc.sync.dma_start(g8[:], in_ap)
                else:
                    for r in range(N_CORES):
                        nc.sync.dma_start(g8[:, :, r], cco[r])
                g = const.tile([C, 2], F32, name=f"g{idx}", tag="sc_g")
                nc.vector.tensor_reduce(g[:], g8[:], mybir.AxisListType.X, OP.add)
                return g

            def conv(pad, ws, dst, stats, i, acol, shcol):
                """3x3 conv of padded +/-1 fp8 image (row pitch 64) -> PReLU'd
                f16 dst [C,HW]. 5 fp8 DoubleRow passes per 448-col tile (pass 4
                pairs the last tap with zero weights). Evacuation fuses PReLU;
                engine alternates ACT/DVE for load balance; bn_stats chunks
                follow each group."""
                padf = pad[:].rearrange("p r w -> p (r w)")
                for g in range(N_GROUPS):
                    tiles = range(g * GTILES, (g + 1) * GTILES)
                    psg = psum.tile([C, GTILES, 512], F32, tag="ps",
                                    name=f"ps{g}", bufs=2)
                    if CONV_MODE == "dr6z":
                        # 6 DoubleRow passes, all rhs pair strides = PITCH
                        # (16B-aligned, HW requirement): 3 vertical tap pairs
                        # (dh 0+1) + 3 (dh=2 tap, zero-slot-9) pairs.
                        wb = ws[:, 0, :]
                        for p_ in range(6):
                            if p_ < 3:
                                woff, wstride, base = p_ * C, 3 * C, TAP_OFF[p_]
                            else:
                                woff = (3 + p_) * C      # slots 6,7,8
                                wstride = (9 - (3 + p_)) * C  # to zero slot 9
                                base = TAP_OFF[3 + p_]
                            wp = bass.AP(wb.tensor, wb.offset + woff,
                                         [list(wb.ap[0]), [wstride, 2], [1, C]])
                            for j, t in enumerate(tiles):
                                q0 = t * QSPAN + base
                                rhs = bass.AP(padf.tensor, padf.offset + q0,
                                              [list(padf.ap[0]), [PITCH, 2],
                                               [1, QSPAN]])
                                nc.tensor.matmul(
                                    psg[:, j, 0:QSPAN], wp, rhs,
                                    start=(p_ == 0), stop=(p_ == 5),
                                    perf_mode=DR,
                                )
                    else:
                        # dr3: vertical pairs (0,dw)+(1,dw) with rhs pair
                        # stride PITCH, then 3 plain passes for dh=2 taps.
                        wb = ws[:, 0, :]
                        for dw in range(3):
                            wp = bass.AP(wb.tensor, wb.offset + dw * C,
                                         [list(wb.ap[0]), [3 * C, 2], [1, C]])
                            for j, t in enumerate(tiles):
                                q0 = t * QSPAN + dw
                                rhs = bass.AP(padf.tensor, padf.offset + q0,
                                              [list(padf.ap[0]), [PITCH, 2],
                                               [1, QSPAN]])
                                nc.tensor.matmul(
                                    psg[:, j, 0:QSPAN], wp, rhs,
                                    start=(dw == 0), stop=False,
                                    perf_mode=DR,
                                )
                        for dw in range(3):
                            for j, t in enumerate(tiles):
                                q0 = t * QSPAN + 2 * PITCH + dw
                                nc.tensor.matmul(
                                    psg[:, j, 0:QSPAN], ws[:, 6 + dw, :],
                                    padf[:, q0 : q0 + QSPAN],
                                    start=False, stop=(dw == 2),
                                )
                    gbase = psg[:]
                    # For the batch's last image, split the final group's
                    # evacuation in half so its bn_stats (which gate the
                    # collective) start one half earlier.
                    nsplit = 2 if (i == N_LOC - 1 and g == N_GROUPS - 1) else 1
                    tper = GTILES // nsplit
                    for h in range(nsplit):
                        src_ap = bass.AP(gbase.tensor,
                                         gbase.offset + h * tper * 512,
                                         [list(gbase.ap[0]), [512, tper],
                                          [PITCH, TILE_ROWS], [1, W]])
                        off = g * GCOMPACT + h * tper * SCHUNK
                        dst_sl = dst[:, off : off + tper * SCHUNK]
                        dst_ap = dst_sl.rearrange("p (t r w) -> p t r w",
                                                  t=tper, r=TILE_ROWS, w=W)
                        nc.scalar.activation(dst_ap, src_ap, AF.Prelu,
                                             alpha=acol, scale=shcol)
                        for k in range(tper):
                            kk = h * tper + k
                            nc.vector.bn_stats(
                                stats[:, i * 8 + g * GTILES + kk, :],
                                dst[:, g * GCOMPACT + kk * SCHUNK
                                       : g * GCOMPACT + (kk + 1) * SCHUNK])

            for _rep in range(reps):
                # ============ Phase A: load x, convert f16, BN1 stats ========
                # Last image loads in quarters so its final bn_stats (which
                # gate the AR1 collective) start a quarter-chunk earlier.
                for i in range(N_LOC):
                    ndma = 1
                    for h in range(2):
                        xin = work.tile([C, HW // 2], F32, tag="xin", bufs=3,
                                        name=f"xa{i}_{h}")
                        qsz = (HW // 2) // ndma
                        for q in range(ndma):
                            nc.sync.dma_start(
                                xin[:, q * qsz : (q + 1) * qsz],
                                x_d[i, :, h * (HW // 2) + q * qsz
                                      : h * (HW // 2) + (q + 1) * qsz])
                        xbsl = xb[:, i, h * (HW // 2) : (h + 1) * (HW // 2)]
                        nc.scalar.activation(xbsl, xin[:], AF.Copy)
                        for k in range(4):
                            # stats from the f32 data: the f16 copy shifts the
                            # BN1 mean enough (~1e-6) to flip near-threshold
                            # signs, which cascades through both binary convs
                            nc.vector.bn_stats(
                                stats1[:, i * 8 + h * 4 + k, :],
                                xin[:, k * SCHUNK : (k + 1) * SCHUNK])

                g1ar = reduce_stats(stats1, 0)
                mean1, rstd1 = _rstd_from_allreduced(nc, const, g1ar, "1")
                k1, c1b, tau1 = _affine_consts(nc, const, pp, mean1, rstd1,
                                               P_G1, P_B1, "1")

                def sign_to_pad(i, src_img, k, cb, tau):
                    """pad interior <- sign(k*src + cb) as +/-1 fp8.
                    DVE (2-op threshold) for some images, ACT for the rest.
                    High priority: the sign gates the PE for this image."""
                    pad = pads[i % N_PADS]
                    dst = pad[:, 1 : H + 1, 1 : W + 1]
                    with tc.high_priority(offset=60):
                        if DVE_SIGN and i in (1, 4, 6):
                            t01 = work.tile([C, HW], F16, tag="d1", bufs=3,
                                            name=f"t01_{i}")
                            nc.vector.tensor_scalar(t01[:], src_img, tau[:],
                                                    None, OP.is_ge)
                            nc.vector.tensor_scalar(
                                dst,
                                t01[:].rearrange("p (h w) -> p h w", h=H, w=W),
                                2.0, -1.0, OP.mult, OP.add)
                        else:
                            nc.scalar.activation(
                                dst,
                                src_img.rearrange("p (h w) -> p h w", h=H, w=W),
                                AF.Sign, bias=cb[:], scale=k[:],
                            )
                    return pad

                # ============ Phase B: b1 = sign(BN1(x)); conv1; stats2 ======
                # sign1 thresholds x near tau1 and errors cascade through two
                # binary convs, so it must read x at full f32 precision:
                # re-load x from HBM (prefetches during phase A / AR1).
                HROWS = H // 2  # 28
                for i in range(N_LOC):
                    pad = pads[i % N_PADS]
                    for h in range(2):
                        # DMA at normal priority (must not preempt phase A's
                        # final loads on the queue); sign ops high priority.
                        xin = work.tile([C, HW // 2], F32, tag="xin",
                                        bufs=3, name=f"xs{i}_{h}")
                        nc.sync.dma_start(
                            xin[:],
                            x_d[i, :, h * (HW // 2) : (h + 1) * (HW // 2)])
                        with tc.high_priority(offset=60):
                            dsth = pad[:, 1 + h * HROWS : 1 + (h + 1) * HROWS,
                                       1 : W + 1]
                            if DVE_SIGN and i in (1, 4, 6):
                                t01 = work.tile([C, HW], F16, tag="d1",
                                                bufs=3, name=f"t01_{i}_{h}")
                                t01h = t01[:, 0 : HW // 2]
                                nc.vector.tensor_scalar(t01h, xin[:], tau1[:],
                                                        None, OP.is_ge)
                                nc.vector.tensor_scalar(
                                    dsth,
                                    t01h.rearrange("p (h w) -> p h w",
                                                   h=HROWS, w=W),
                                    2.0, -1.0, OP.mult, OP.add)
                            else:
                                nc.scalar.activation(
                                    dsth,
                                    xin[:].rearrange("p (h w) -> p h w",
                                                     h=HROWS, w=W),
                                    AF.Sign, bias=c1b[:], scale=k1[:],
                                )
                    if dbg and i == 0:
                        nc.sync.dma_start(dbg_pad_d[:], pad[:])
                    conv(pad, w1s, c1f[:, i, :], stats2, i, a1, s1h)

                g2ar = reduce_stats(stats2, 1, r1, r1sq[:])
                mean2, rstd2 = _rstd_from_allreduced(nc, const, g2ar, "2")
                k2, c2b, tau2 = _affine_consts(nc, const, pp, mean2, rstd2,
                                               P_G2, P_B2, "2", rho=r1)

                # ============ Phase C: b2 = sign(BN2(p1)); conv2; stats3 =====
                for i in range(N_LOC):
                    pad = sign_to_pad(i, c1f[:, i, :], k2, c2b, tau2)
                    conv(pad, w2s, c2f[:, i, :], stats3, i, a2, s2h)

                g3ar = reduce_stats(stats3, 2, r2, r2sq[:])
                mean3, rstd3 = _rstd_from_allreduced(nc, const, g3ar, "3")
                k3, c3b, _tau3 = _affine_consts(nc, const, pp, mean3, rstd3,
                                                P_G3, P_B3, "3", rho=r2)

                if dbg:
                    nc.sync.dma_start(dbg_c1_d[:], c1f[:])
                    nc.sync.dma_start(dbg_c2_d[:], c2f[:])
                    dbgk = const.tile([C, 8], F32)
                    for j, t_ in enumerate(
                        [k1, c1b, tau1, k2, c2b, tau2, k3, c3b]
                    ):
                        nc.vector.tensor_copy(_col(dbgk, j), t_[:])
                    nc.sync.dma_start(dbg_k_d[:], dbgk[:])

                # ====== Phase D: y = PReLU(k3*p2 + x + c3b) ======
                for i in range(N_LOC):
                    d1 = work.tile([C, HW], F16, tag="d1", bufs=3)
                    nc.vector.tensor_scalar(d1[:], c2f[:, i, :], k3[:],
                                            None, OP.mult)
                    nc.vector.tensor_tensor(out=d1[:], in0=d1[:],
                                            in1=xb[:, i, :], op=OP.add)
                    if i >= N_LOC - 2:
                        # halve the trailing prelu+DMA chain of the last images
                        for h in range(2):
                            sl = slice(h * (HW // 2), (h + 1) * (HW // 2))
                            nc.scalar.activation(c1f[:, i, sl], d1[:, sl],
                                                 AF.Prelu, bias=c3b[:],
                                                 alpha=a3)
                            nc.sync.dma_start(out_d[i][:, sl], c1f[:, i, sl])
                    else:
                        nc.scalar.activation(c1f[:, i, :], d1[:], AF.Prelu,
                                             bias=c3b[:], alpha=a3)
                        nc.sync.dma_start(out_d[i], c1f[:, i, :])

    nc.compile()
    return nc


def _prep_host(x, bn1_g, bn1_b, w1, prelu1_a, bn2_g, bn2_b, w2, prelu2_a,
               bn3_g, bn3_b, prelu3_a):
    def wprep(w_flat):
        w = np.asarray(w_flat, np.float32).reshape(C, C, 3, 3)
        # lhsT layout [i, slot, o] = sign(w[o, i, dh, dw]); slot 9 zero-padded
        wT = np.sign(w).transpose(1, 2, 3, 0).reshape(C, 9, C)
        w10 = np.zeros((C, 10, C), np.float32)
        w10[:, :9, :] = wT
        s = np.mean(np.abs(w), axis=(1, 2, 3)).astype(np.float32)  # [C] per o
        s_hat = np.exp2(np.round(np.log2(s))).astype(np.float32)
        rho = (s / s_hat).astype(np.float32)
        return w10.astype(mybir.dt.np(FP8)), s_hat, rho

    w1t, s1h_, r1_ = wprep(w1)
    w2t, s2h_, r2_ = wprep(w2)

    pp = np.zeros((C, NP), np.float32)
    pp[:, P_S1H] = s1h_
    pp[:, P_R1] = r1_
    pp[:, P_S2H] = s2h_
    pp[:, P_R2] = r2_
    pp[:, P_G1] = np.asarray(bn1_g, np.float32)
    pp[:, P_B1] = np.asarray(bn1_b, np.float32)
    pp[:, P_G2] = np.asarray(bn2_g, np.float32)
    pp[:, P_B2] = np.asarray(bn2_b, np.float32)
    pp[:, P_G3] = np.asarray(bn3_g, np.float32)
    pp[:, P_B3] = np.asarray(bn3_b, np.float32)
    pp[:, P_A1] = np.float32(prelu1_a)
    pp[:, P_A2] = np.float32(prelu2_a)
    pp[:, P_A3] = np.float32(prelu3_a)

    x = np.ascontiguousarray(np.asarray(x, np.float32).reshape(64, C, HW))
    in_maps = []
    for r in range(N_CORES):
        in_maps.append({
            "x": x[r * N_LOC : (r + 1) * N_LOC],
            "w1t": w1t,
            "w2t": w2t,
            "pp": pp,
        })
    return in_maps


_NC_CACHE = None


def _get_nc():
    global _NC_CACHE
    if _NC_CACHE is None:
        _NC_CACHE = build_nc()
    return _NC_CACHE


def run(in_maps, **kwargs):
    nc = _get_nc()
    return run_bass_kernel_spmd(nc, in_maps, core_ids=list(range(N_CORES)), **kwargs)


def kernel(**inputs):
    in_maps = _prep_host(**inputs)
    last_err = None
    for attempt in range(3):
        try:
            res = run(in_maps)
            break
        except Exception as e:  # transient NRT device errors happen; retry
            last_err = e
            import time as _time
            _time.sleep(2.0)
    else:
        raise last_err
    out = np.concatenate(
        [np.asarray(r["out"]).astype(np.float32).reshape(N_LOC, C, H, W)
         for r in res.results], axis=0
    )
    return out


if __name__ == "__main__":
    rng = np.random.default_rng(0)
    x = rng.standard_normal((64, C, H, W), dtype=np.float32)
    w1 = ((rng.random((C * C * 9, 1), dtype=np.float32) - 0.5) * 0.002)
    w2 = ((rng.random((C * C * 9, 1), dtype=np.float32) - 0.5) * 0.002)
    ones = np.ones(C, np.float32)
    zeros = np.zeros(C, np.float32)
    y = kernel(x=x, bn1_g=ones, bn1_b=zeros, w1=w1, prelu1_a=np.float32(0.25),
               bn2_g=ones, bn2_b=zeros, w2=w2, prelu2_a=np.float32(0.25),
               bn3_g=ones, bn3_b=zeros, prelu3_a=np.float32(0.25))
    print("out", y.shape, y.dtype, float(np.abs(y).mean()))


# revision 44
# speedup vs baseline: 1.0014x; 1.0013x over previous
"""XNOR-Net BasicBlock forward (BN-sign-binconv-PReLU x2 + BN + residual + PReLU)
distributed over 8 Trainium2 NeuronCores, data-parallel over the batch axis.

Self-contained: hardcodes shapes N=64, C=128, H=W=56, 8 cores.

Design notes:
- Phase A: x streams from HBM (f32) feeding BN1 bn_stats (full f32 precision:
  the sign1 threshold cascades through both binary convs, so stats from a
  rounded copy flip near-threshold pixels) and an f16 copy (xb) kept in SBUF
  for the phase-D residual. Phase B re-loads x (f32) for the sign itself.
- 3x3 binary conv: 6 fp8 DoubleRow matmul passes per 448-col PSUM tile — 3
  vertical tap pairs (dh 0+1) plus 3 (dh=2 tap, zero-weight slot) pairs. All
  rhs pair strides = PITCH = 64 (DoubleRow requires 16-aligned pair strides
  on HW; unaligned strides wedge the PE). 3 PE cycles/pixel.
- PSUM evacuation is a fused ACT Prelu with scale = s_hat (the power-of-2
  rounding of the conv scale s = mean|w|, so p' = s_hat*prelu(c) is exact in
  f16). BN's var+EPS is NOT scale-invariant (s ~ 5e-4 makes var comparable to
  EPS), so stats and thresholds are corrected by rho = s/s_hat per channel.
- sign ops: ACT Sign(scale, bias) for most images; a 2-op DVE threshold
  (is_ge + affine to fp8) for some, to balance engine load.
- BN batch stats: bn_stats/bn_aggr per core, AllGather of [C,2] partial
  moments, single strided gather DMA, on-chip reduce, Newton-refined rsqrt.
"""

import numpy as np
import ml_dtypes

import concourse.bass as bass
import concourse.mybir as mybir
import concourse.tile as tile
from concourse import bacc
from concourse.bass_utils import run_bass_kernel_spmd

F32 = mybir.dt.float32
F16 = mybir.dt.float16
FP8 = mybir.dt.float8e4
PITCH = 64
AF = mybir.ActivationFunctionType
OP = mybir.AluOpType

N_CORES = 8
N_LOC = 8          # images per core
C = 128            # channels (== partitions)
H = W = 56
HW = H * W         # 3136
EPS = 1e-5
PROWS = H + 4      # padded rows + 2 spare (garbage-col / zero-pair overreads)
TILE_ROWS = 7
N_TILES = H // TILE_ROWS     # 8
QSPAN = TILE_ROWS * PITCH    # 448 padded cols per tile
GTILES = 4                   # tiles per PSUM group (4 banks)
N_GROUPS = N_TILES // GTILES # 2
GCOMPACT = GTILES * TILE_ROWS * W   # 1568 compact elems per group
SCHUNK = TILE_ROWS * W       # 392, bn_stats chunk (4 per group)

# flat padded offsets of the 9 taps (dh*PITCH + dw), natural order; pairs are
# (0,1) (2,3) (4,5) (6,7), single tap 8 is paired with zero weights (slot 9)
TAP_OFF = [0, 1, 2, PITCH, PITCH + 1, PITCH + 2, 2 * PITCH, 2 * PITCH + 1,
           2 * PITCH + 2]
PAIR_D = [TAP_OFF[1] - TAP_OFF[0], TAP_OFF[3] - TAP_OFF[2],
          TAP_OFF[5] - TAP_OFF[4], TAP_OFF[7] - TAP_OFF[6], 1]

# pp param columns. S1H/S2H: power-of-2 rounding of the conv scales s=mean|w|
# (exact in f16); R1/R2: residual ratio rho = s/s_hat, applied to stats and
# sign thresholds so BN's var+EPS sees the reference-scaled values.
(P_G1, P_B1, P_G2, P_B2, P_G3, P_B3, P_A1, P_A2, P_A3,
 P_S1H, P_R1, P_S2H, P_R2) = range(13)
NP = 13

DR = mybir.MatmulPerfMode.DoubleRow


def _col(t, j):
    return t[:, j : j + 1]


def _rstd_from_allreduced(nc, pool, ar, name):
    """ar: [128,2] = sum over cores of [mean_i, var_i + mean_i^2].
    Returns (mean, rstd) tiles [128,1] f32 with rstd = 1/sqrt(var+EPS),
    Newton-refined to cover ScalarE Sqrt spline error."""
    mean = pool.tile([C, 1], F32, name=f"mean_{name}", tag=f"mean_{name}")
    ex2 = pool.tile([C, 1], F32, name=f"ex2_{name}", tag="sc_ex2")
    nc.vector.tensor_scalar_mul(mean[:], _col(ar, 0), 1.0 / N_CORES)
    nc.vector.tensor_scalar_mul(ex2[:], _col(ar, 1), 1.0 / N_CORES)
    negmean = pool.tile([C, 1], F32, name=f"negmean_{name}", tag="sc_negmean")
    nc.vector.tensor_scalar_mul(negmean[:], mean[:], -1.0)
    vpe = pool.tile([C, 1], F32, name=f"vpe_{name}", tag="sc_vpe")
    nc.vector.scalar_tensor_tensor(vpe[:], mean[:], negmean[:], ex2[:], OP.mult, OP.add)
    nc.vector.tensor_scalar_add(vpe[:], vpe[:], EPS)
    rec = pool.tile([C, 1], F32, name=f"rec_{name}", tag="sc_rec")
    nc.vector.reciprocal(rec[:], vpe[:])
    rstd = pool.tile([C, 1], F32, name=f"rstd_{name}", tag=f"rstd_{name}")
    nc.scalar.activation(rstd[:], rec[:], AF.Sqrt)
    # Newton: y <- y * (1.5 - 0.5 * vpe * y^2)
    t1 = pool.tile([C, 1], F32, name=f"t1_{name}", tag="sc_t1")
    nc.vector.tensor_tensor(out=t1[:], in0=rstd[:], in1=rstd[:], op=OP.mult)
    nc.vector.tensor_tensor(out=t1[:], in0=t1[:], in1=vpe[:], op=OP.mult)
    nc.vector.tensor_scalar(t1[:], t1[:], -0.5, 1.5, OP.mult, OP.add)
    nc.vector.tensor_tensor(out=rstd[:], in0=rstd[:], in1=t1[:], op=OP.mult)
    return mean, rstd


def _affine_consts(nc, pool, pp, mean, rstd, g_col, b_col, name, rho=None):
    """k = g * rstd (in reference units); cb = b - mean * k.
    If rho is given, the consumer reads the s_hat-scaled tensor, so the
    returned slope is ks = k * rho. tau = -cb/ks is the threshold in
    consumer-input units (valid for ks > 0)."""
    k = pool.tile([C, 1], F32, name=f"k_{name}", tag=f"k_{name}")
    nc.vector.tensor_tensor(out=k[:], in0=_col(pp, g_col), in1=rstd[:], op=OP.mult)
    if rho is not None:
        nc.vector.tensor_tensor(out=k[:], in0=k[:], in1=rho, op=OP.mult)
        # cb must use the reference-unit slope: cb = b - mean_ref * k_ref,
        # and mean passed in is already reference-unit, so recompute k_ref
        # separately? No: mean_ref * k_ref == (mean_ref) * (ks / rho).
        # Simpler: cb = b - (mean_ref / rho) * ks. Precompute mratio.
    negk = pool.tile([C, 1], F32, name=f"negk_{name}", tag="sc_negk")
    nc.vector.tensor_scalar_mul(negk[:], k[:], -1.0)
    cb = pool.tile([C, 1], F32, name=f"cb_{name}", tag=f"cb_{name}")
    if rho is None:
        nc.vector.scalar_tensor_tensor(
            cb[:], mean[:], negk[:], _col(pp, b_col), OP.mult, OP.add
        )
    else:
        # mean here is reference-unit; consumer-unit mean is mean/rho
        mc = pool.tile([C, 1], F32, name=f"mc_{name}", tag="sc_mc")
        rrho = pool.tile([C, 1], F32, name=f"rrho_{name}", tag="sc_rrho")
        nc.vector.reciprocal(rrho[:], rho)
        nc.vector.tensor_tensor(out=mc[:], in0=mean[:], in1=rrho[:], op=OP.mult)
        nc.vector.scalar_tensor_tensor(
            cb[:], mc[:], negk[:], _col(pp, b_col), OP.mult, OP.add
        )
    # tau = -cb/ks (threshold in consumer-input units)
    rk = pool.tile([C, 1], F32, name=f"rk_{name}", tag="sc_rk")
    nc.vector.reciprocal(rk[:], k[:])
    tau = pool.tile([C, 1], F32, name=f"tau_{name}", tag=f"tau_{name}")
    nc.vector.tensor_tensor(out=tau[:], in0=cb[:], in1=rk[:], op=OP.mult)
    nc.vector.tensor_scalar_mul(tau[:], tau[:], -1.0)
    return k, cb, tau


import os

CONV_MODE = os.environ.get("K_CONV_MODE", "dr6z")  # dr6z | dr3
GATHER1 = os.environ.get("K_GATHER1", "1") == "1"  # single gather DMA
DVE_SIGN = os.environ.get("K_DVE_SIGN", "1") == "1"


def build_nc(reps=1, dbg=False):
    nc = bacc.Bacc(None, target_bir_lowering=False, debug=False, num_devices=N_CORES)

    x_d = nc.dram_tensor("x", [N_LOC, C, HW], F32, kind="ExternalInput")
    w1_d = nc.dram_tensor("w1t", [C, 10, C], FP8, kind="ExternalInput")
    w2_d = nc.dram_tensor("w2t", [C, 10, C], FP8, kind="ExternalInput")
    pp_d = nc.dram_tensor("pp", [C, NP], F32, kind="ExternalInput")
    out_d = nc.dram_tensor("out", [N_LOC, C, HW], F16, kind="ExternalOutput")
    if dbg:
        dbg_pad_d = nc.dram_tensor("dbg_pad", [C, PROWS, PITCH], FP8,
                                   kind="ExternalOutput")
        dbg_c1_d = nc.dram_tensor("dbg_c1", [C, N_LOC, HW], F16,
                                  kind="ExternalOutput")
        dbg_c2_d = nc.dram_tensor("dbg_c2", [C, N_LOC, HW], F16,
                                  kind="ExternalOutput")
        dbg_k_d = nc.dram_tensor("dbg_k", [C, 8], F32, kind="ExternalOutput")

    with tile.TileContext(nc) as tc:
        with (
            tc.tile_pool(name="const", bufs=1) as const,
            tc.tile_pool(name="work", bufs=2) as work,
            tc.tile_pool(name="psum", bufs=2, space="PSUM") as psum,
            tc.tile_pool(name="dram", bufs=1, space="DRAM") as dram,
        ):
            # ---- persistent SBUF tensors ----
            pp = const.tile([C, NP], F32)
            nc.gpsimd.dma_start(pp[:], pp_d[:])
            w1s = const.tile([C, 10, C], FP8)
            w2s = const.tile([C, 10, C], FP8)
            nc.gpsimd.dma_start(w1s[:], w1_d[:])
            nc.gpsimd.dma_start(w2s[:], w2_d[:])
            xb = const.tile([C, N_LOC, HW], F16)    # f16 copy of x
            c1f = const.tile([C, N_LOC, HW], F16)   # p1 = prelu_a1(c1)
            c2f = const.tile([C, N_LOC, HW], F16)   # p2 = prelu_a2(c2)
            stats1 = const.tile([C, N_LOC * 8, 6], F32, tag="stats1")
            stats2 = const.tile([C, N_LOC * 8, 6], F32, tag="stats2")
            stats3 = const.tile([C, N_LOC * 8, 6], F32, tag="stats3")
            N_PADS = 3
            pads = []
            for j in range(N_PADS):
                p = const.tile([C, PROWS, PITCH], FP8, name=f"pad{j}")
                nc.gpsimd.memset(p[:], 0.0)
                pads.append(p)

            a1 = _col(pp, P_A1)
            a2 = _col(pp, P_A2)
            a3 = _col(pp, P_A3)
            s1h = _col(pp, P_S1H)
            s2h = _col(pp, P_S2H)
            r1 = _col(pp, P_R1)
            r2 = _col(pp, P_R2)
            r1sq = const.tile([C, 1], F32, name="r1sq")
            nc.vector.tensor_tensor(out=r1sq[:], in0=r1, in1=r1, op=OP.mult)
            r2sq = const.tile([C, 1], F32, name="r2sq")
            nc.vector.tensor_tensor(out=r2sq[:], in0=r2, in1=r2, op=OP.mult)

            cc_counter = [0]

            def reduce_stats(stats, idx, rho=None, rhosq=None):
                """bn_aggr + pack [mean, var+mean^2] (rescaled into reference
                units by rho) + allgather-sum. Returns [128,2] tile of global
                [sum mean_i, sum (var_i+m_i^2)]."""
                mv = const.tile([C, 2], F32, name=f"mv{idx}", tag="sc_mv")
                nc.vector.bn_aggr(mv[:], stats[:])
                e = const.tile([C, 2], F32, name=f"e{idx}", tag="sc_e")
                if rho is None:
                    nc.vector.tensor_copy(_col(e, 0), _col(mv, 0))
                else:
                    nc.vector.tensor_tensor(out=_col(e, 0), in0=_col(mv, 0),
                                            in1=rho, op=OP.mult)
                nc.vector.scalar_tensor_tensor(
                    _col(e, 1), _col(mv, 0), _col(mv, 0), _col(mv, 1), OP.mult, OP.add
                )
                if rhosq is not None:
                    nc.vector.tensor_tensor(out=_col(e, 1), in0=_col(e, 1),
                                            in1=rhosq, op=OP.mult)
                n = cc_counter[0]
                cc_counter[0] += 1
                cci = dram.tile([C, 2], F32, name=f"cc_in{n}", tag=f"cc_in{n}")
                cco = dram.tile([N_CORES, C, 2], F32, name=f"cc_out{n}",
                                tag=f"cc_out{n}", addr_space="Shared")
                nc.sync.dma_start(cci[:], e[:])
                nc.gpsimd.collective_compute(
                    "AllGather",
                    OP.bypass,
                    replica_groups=[list(range(N_CORES))],
                    ins=[cci.opt()],
                    outs=[cco.opt()],
                )
                # gather: g8[c, j, r] <- cco[r, c, j]
                g8 = const.tile([C, 2, N_CORES], F32, name=f"g8{idx}", tag="sc_g8")
                if GATHER1:
                    c0 = cco[0]
                    in_ap = bass.AP(c0.tensor, c0.offset,
                                    [list(c0.ap[0]), [1, 2], [2 * C, N_CORES]])
                    nc.sync.dma_start(g8[:], in_ap)
                else:
                    for r in range(N_CORES):
                        nc.sync.dma_start(g8[:, :, r], cco[r])
                g = const.tile([C, 2], F32, name=f"g{idx}", tag="sc_g")
                nc.vector.tensor_reduce(g[:], g8[:], mybir.AxisListType.X, OP.add)
                return g

            def conv(pad, ws, dst, stats, i, acol, shcol):
                """3x3 conv of padded +/-1 fp8 image (row pitch 64) -> PReLU'd
                f16 dst [C,HW]. 5 fp8 DoubleRow passes per 448-col tile (pass 4
                pairs the last tap with zero weights). Evacuation fuses PReLU;
                engine alternates ACT/DVE for load balance; bn_stats chunks
                follow each group."""
                padf = pad[:].rearrange("p r w -> p (r w)")
                for g in range(N_GROUPS):
                    tiles = range(g * GTILES, (g + 1) * GTILES)
                    psg = psum.tile([C, GTILES, 512], F32, tag="ps",
                                    name=f"ps{g}", bufs=2)
                    if CONV_MODE == "dr6z":
                        # 6 DoubleRow passes, all rhs pair strides = PITCH
                        # (16B-aligned, HW requirement): 3 vertical tap pairs
                        # (dh 0+1) + 3 (dh=2 tap, zero-slot-9) pairs.
                        wb = ws[:, 0, :]
                        for p_ in range(6):
                            if p_ < 3:
                                woff, wstride, base = p_ * C, 3 * C, TAP_OFF[p_]
                            else:
                                woff = (3 + p_) * C      # slots 6,7,8
                                wstride = (9 - (3 + p_)) * C  # to zero slot 9
                                base = TAP_OFF[3 + p_]
                            wp = bass.AP(wb.tensor, wb.offset + woff,
                                         [list(wb.ap[0]), [wstride, 2], [1, C]])
                            for j, t in enumerate(tiles):
                                q0 = t * QSPAN + base
                                rhs = bass.AP(padf.tensor, padf.offset + q0,
                                              [list(padf.ap[0]), [PITCH, 2],
                                               [1, QSPAN]])
                                nc.tensor.matmul(
                                    psg[:, j, 0:QSPAN], wp, rhs,
                                    start=(p_ == 0), stop=(p_ == 5),
                                    perf_mode=DR,
                                )
                    else:
                        # dr3: vertical pairs (0,dw)+(1,dw) with rhs pair
                        # stride PITCH, then 3 plain passes for dh=2 taps.
                        wb = ws[:, 0, :]
                        for dw in range(3):
                            wp = bass.AP(wb.tensor, wb.offset + dw * C,
                                         [list(wb.ap[0]), [3 * C, 2], [1, C]])
                            for j, t in enumerate(tiles):
                                q0 = t * QSPAN + dw
                                rhs = bass.AP(padf.tensor, padf.offset + q0,
                                              [list(padf.ap[0]), [PITCH, 2],
                                               [1, QSPAN]])
                                nc.tensor.matmul(
                                    psg[:, j, 0:QSPAN], wp, rhs,
                                    start=(dw == 0), stop=False,
                                    perf_mode=DR,
                                )
                        for dw in range(3):
                            for j, t in enumerate(tiles):
                                q0 = t * QSPAN + 2 * PITCH + dw
                                nc.tensor.matmul(
                                    psg[:, j, 0:QSPAN], ws[:, 6 + dw, :],
                                    padf[:, q0 : q0 + QSPAN],
                                    start=False, stop=(dw == 2),
                                )
                    gbase = psg[:]
                    # For the batch's last image, split the final group's
                    # evacuation in half so its bn_stats (which gate the
                    # collective) start one half earlier.
                    nsplit = 2 if (i == N_LOC - 1 and g == N_GROUPS - 1) else 1
                    tper = GTILES // nsplit
                    for h in range(nsplit):
                        src_ap = bass.AP(gbase.tensor,
                                         gbase.offset + h * tper * 512,
                                         [list(gbase.ap[0]), [512, tper],
                                          [PITCH, TILE_ROWS], [1, W]])
                        off = g * GCOMPACT + h * tper * SCHUNK
                        dst_sl = dst[:, off : off + tper * SCHUNK]
                        dst_ap = dst_sl.rearrange("p (t r w) -> p t r w",
                                                  t=tper, r=TILE_ROWS, w=W)
                        nc.scalar.activation(dst_ap, src_ap, AF.Prelu,
                                             alpha=acol, scale=shcol)
                        for k in range(tper):
                            kk = h * tper + k
                            nc.vector.bn_stats(
                                stats[:, i * 8 + g * GTILES + kk, :],
                                dst[:, g * GCOMPACT + kk * SCHUNK
                                       : g * GCOMPACT + (kk + 1) * SCHUNK])

            for _rep in range(reps):
                # ============ Phase A: load x, convert f16, BN1 stats ========
                # Last image loads in quarters so its final bn_stats (which
                # gate the AR1 collective) start a quarter-chunk earlier.
                for i in range(N_LOC):
                    ndma = 1
                    for h in range(2):
                        xin = work.tile([C, HW // 2], F32, tag="xin", bufs=3,
                                        name=f"xa{i}_{h}")
                        qsz = (HW // 2) // ndma
                        for q in range(ndma):
                            nc.sync.dma_start(
                                xin[:, q * qsz : (q + 1) * qsz],
                                x_d[i, :, h * (HW // 2) + q * qsz
                                      : h * (HW // 2) + (q + 1) * qsz])
                        xbsl = xb[:, i, h * (HW // 2) : (h + 1) * (HW // 2)]
                        nc.scalar.activation(xbsl, xin[:], AF.Copy)
                        for k in range(4):
                            # stats from the f32 data: the f16 copy shifts the
                            # BN1 mean enough (~1e-6) to flip near-threshold
                            # signs, which cascades through both binary convs
                            nc.vector.bn_stats(
                                stats1[:, i * 8 + h * 4 + k, :],
                                xin[:, k * SCHUNK : (k + 1) * SCHUNK])

                g1ar = reduce_stats(stats1, 0)
                mean1, rstd1 = _rstd_from_allreduced(nc, const, g1ar, "1")
                k1, c1b, tau1 = _affine_consts(nc, const, pp, mean1, rstd1,
                                               P_G1, P_B1, "1")

                def sign_to_pad(i, src_img, k, cb, tau):
                    """pad interior <- sign(k*src + cb) as +/-1 fp8.
                    DVE (2-op threshold) for some images, ACT for the rest.
                    High priority: the sign gates the PE for this image."""
                    pad = pads[i % N_PADS]
                    dst = pad[:, 1 : H + 1, 1 : W + 1]
                    with tc.high_priority(offset=60):
                        if DVE_SIGN and i in (1, 4, 6):
                            t01 = work.tile([C, HW], F16, tag="d1", bufs=3,
                                            name=f"t01_{i}")
                            nc.vector.tensor_scalar(t01[:], src_img, tau[:],
                                                    None, OP.is_ge)
                            nc.vector.tensor_scalar(
                                dst,
                                t01[:].rearrange("p (h w) -> p h w", h=H, w=W),
                                2.0, -1.0, OP.mult, OP.add)
                        else:
                            nc.scalar.activation(
                                dst,
                                src_img.rearrange("p (h w) -> p h w", h=H, w=W),
                                AF.Sign, bias=cb[:], scale=k[:],
                            )
                    return pad

                # ============ Phase B: b1 = sign(BN1(x)); conv1; stats2 ======
                # sign1 thresholds x near tau1 and errors cascade through two
                # binary convs, so it must read x at full f32 precision:
                # re-load x from HBM (prefetches during phase A / AR1).
                HROWS = H // 2  # 28
                for i in range(N_LOC):
                    pad = pads[i % N_PADS]
                    for h in range(2):
                        with tc.high_priority(offset=60):
                            xin = work.tile([C, HW // 2], F32, tag="xin",
                                            bufs=3, name=f"xs{i}_{h}")
                            nc.sync.dma_start(
                                xin[:],
                                x_d[i, :, h * (HW // 2) : (h + 1) * (HW // 2)])
                            dsth = pad[:, 1 + h * HROWS : 1 + (h + 1) * HROWS,
                                       1 : W + 1]
                            if DVE_SIGN and i in (1, 4, 6):
                                t01 = work.tile([C, HW], F16, tag="d1",
                                                bufs=3, name=f"t01_{i}_{h}")
                                t01h = t01[:, 0 : HW // 2]
                                nc.vector.tensor_scalar(t01h, xin[:], tau1[:],
                                                        None, OP.is_ge)
                                nc.vector.tensor_scalar(
                                    dsth,
                                    t01h.rearrange("p (h w) -> p h w",
                                                   h=HROWS, w=W),
                                    2.0, -1.0, OP.mult, OP.add)
                            else:
                                nc.scalar.activation(
                                    dsth,
                                    xin[:].rearrange("p (h w) -> p h w",
                                                     h=HROWS, w=W),
                                    AF.Sign, bias=c1b[:], scale=k1[:],
                                )
                    if dbg and i == 0:
                        nc.sync.dma_start(dbg_pad_d[:], pad[:])
                    conv(pad, w1s, c1f[:, i, :], stats2, i, a1, s1h)

                g2ar = reduce_stats(stats2, 1, r1, r1sq[:])
                mean2, rstd2 = _rstd_from_allreduced(nc, const, g2ar, "2")
                k2, c2b, tau2 = _affine_consts(nc, const, pp, mean2, rstd2,
                                               P_G2, P_B2, "2", rho=r1)

                # ============ Phase C: b2 = sign(BN2(p1)); conv2; stats3 =====
                for i in range(N_LOC):
                    pad = sign_to_pad(i, c1f[:, i, :], k2, c2b, tau2)
                    conv(pad, w2s, c2f[:, i, :], stats3, i, a2, s2h)

                g3ar = reduce_stats(stats3, 2, r2, r2sq[:])
                mean3, rstd3 = _rstd_from_allreduced(nc, const, g3ar, "3")
                k3, c3b, _tau3 = _affine_consts(nc, const, pp, mean3, rstd3,
                                                P_G3, P_B3, "3", rho=r2)

                if dbg:
                    nc.sync.dma_start(dbg_c1_d[:], c1f[:])
                    nc.sync.dma_start(dbg_c2_d[:], c2f[:])
                    dbgk = const.tile([C, 8], F32)
                    for j, t_ in enumerate(
                        [k1, c1b, tau1, k2, c2b, tau2, k3, c3b]
                    ):
                        nc.vector.tensor_copy(_col(dbgk, j), t_[:])
                    nc.sync.dma_start(dbg_k_d[:], dbgk[:])

                # ====== Phase D: y = PReLU(k3*p2 + x + c3b) ======
                for i in range(N_LOC):
                    d1 = work.tile([C, HW], F16, tag="d1", bufs=3)
                    nc.vector.tensor_scalar(d1[:], c2f[:, i, :], k3[:],
                                            None, OP.mult)
                    nc.vector.tensor_tensor(out=d1[:], in0=d1[:],
                                            in1=xb[:, i, :], op=OP.add)
                    if i >= N_LOC - 2:
                        # halve the trailing prelu+DMA chain of the last images
                        for h in range(2):
                            sl = slice(h * (HW // 2), (h + 1) * (HW // 2))
                            nc.scalar.activation(c1f[:, i, sl], d1[:, sl],
                                                 AF.Prelu, bias=c3b[:],
                                                 alpha=a3)
                            nc.sync.dma_start(out_d[i][:, sl], c1f[:, i, sl])
                    else:
                        nc.scalar.activation(c1f[:, i, :], d1[:], AF.Prelu,
                                             bias=c3b[:], alpha=a3)
                        nc.sync.dma_start(out_d[i], c1f[:, i, :])

    nc.compile()
    return nc


def _prep_host(x, bn1_g, bn1_b, w1, prelu1_a, bn2_g, bn2_b, w2, prelu2_a,
               bn3_g, bn3_b, prelu3_a):
    def wprep(w_flat):
        w = np.asarray(w_flat, np.float32).reshape(C, C, 3, 3)
        # lhsT layout [i, slot, o] = sign(w[o, i, dh, dw]); slot 9 zero-padded
        wT = np.sign(w).transpose(1, 2, 3, 0).reshape(C, 9, C)
        w10 = np.zeros((C, 10, C), np.float32)
        w10[:, :9, :] = wT
        s = np.mean(np.abs(w), axis=(1, 2, 3)).astype(np.float32)  # [C] per o
        s_hat = np.exp2(np.round(np.log2(s))).astype(np.float32)
        rho = (s / s_hat).astype(np.float32)
        return w10.astype(mybir.dt.np(FP8)), s_hat, rho

    w1t, s1h_, r1_ = wprep(w1)
    w2t, s2h_, r2_ = wprep(w2)

    pp = np.zeros((C, NP), np.float32)
    pp[:, P_S1H] = s1h_
    pp[:, P_R1] = r1_
    pp[:, P_S2H] = s2h_
    pp[:, P_R2] = r2_
    pp[:, P_G1] = np.asarray(bn1_g, np.float32)
    pp[:, P_B1] = np.asarray(bn1_b, np.float32)
    pp[:, P_G2] = np.asarray(bn2_g, np.float32)
    pp[:, P_B2] = np.asarray(bn2_b, np.float32)
    pp[:, P_G3] = np.asarray(bn3_g, np.float32)
    pp[:, P_B3] = np.asarray(bn3_b, np.float32)
    pp[:, P_A1] = np.float32(prelu1_a)
    pp[:, P_A2] = np.float32(prelu2_a)
    pp[:, P_A3] = np.float32(prelu3_a)

    x = np.ascontiguousarray(np.asarray(x, np.float32).reshape(64, C, HW))
    in_maps = []
    for r in range(N_CORES):
        in_maps.append({
            "x": x[r * N_LOC : (r + 1) * N_LOC],
            "w1t": w1t,
            "w2t": w2t,
            "pp": pp,
        })
    return in_maps


_NC_CACHE = None


def _get_nc():
    global _NC_CACHE
    if _NC_CACHE is None:
        _NC_CACHE = build_nc()
    return _NC_CACHE


def run(in_maps, **kwargs):
    nc = _get_nc()
    return run_bass_kernel_spmd(nc, in_maps, core_ids=list(range(N_CORES)), **kwargs)


def kernel(**inputs):
    in_maps = _prep_host(**inputs)
    last_err = None
    for attempt in range(3):
        try:
            res = run(in_maps)
            break
        except Exception as e:  # transient NRT device errors happen; retry
            last_err = e
            import time as _time
            _time.sleep(2.0)
    else:
        raise last_err
    out = np.concatenate(
        [np.asarray(r["out"]).astype(np.float32).reshape(N_LOC, C, H, W)
         for r in res.results], axis=0
    )
    return out


if __name__ == "__main__":
    rng = np.random.default_rng(0)
    x = rng.standard_normal((64, C, H, W), dtype=np.float32)
    w1 = ((rng.random((C * C * 9, 1), dtype=np.float32) - 0.5) * 0.002)
    w2 = ((rng.random((C * C * 9, 1), dtype=np.float32) - 0.5) * 0.002)
    ones = np.ones(C, np.float32)
    zeros = np.zeros(C, np.float32)
    y = kernel(x=x, bn1_g=ones, bn1_b=zeros, w1=w1, prelu1_a=np.float32(0.25),
               bn2_g=ones, bn2_b=zeros, w2=w2, prelu2_a=np.float32(0.25),
               bn3_g=ones, bn3_b=zeros, prelu3_a=np.float32(0.25))
    print("out", y.shape, y.dtype, float(np.abs(y).mean()))


# revision 47
# speedup vs baseline: 1.0037x; 1.0023x over previous
"""XNOR-Net BasicBlock forward (BN-sign-binconv-PReLU x2 + BN + residual + PReLU)
distributed over 8 Trainium2 NeuronCores, data-parallel over the batch axis.

Self-contained: hardcodes shapes N=64, C=128, H=W=56, 8 cores.

Design notes:
- Phase A: x streams from HBM (f32) feeding BN1 bn_stats (full f32 precision:
  the sign1 threshold cascades through both binary convs, so stats from a
  rounded copy flip near-threshold pixels) and an f16 copy (xb) kept in SBUF
  for the phase-D residual. Phase B re-loads x (f32) for the sign itself.
- 3x3 binary conv: 6 fp8 DoubleRow matmul passes per 448-col PSUM tile — 3
  vertical tap pairs (dh 0+1) plus 3 (dh=2 tap, zero-weight slot) pairs. All
  rhs pair strides = PITCH = 64 (DoubleRow requires 16-aligned pair strides
  on HW; unaligned strides wedge the PE). 3 PE cycles/pixel.
- PSUM evacuation is a fused ACT Prelu with scale = s_hat (the power-of-2
  rounding of the conv scale s = mean|w|, so p' = s_hat*prelu(c) is exact in
  f16). BN's var+EPS is NOT scale-invariant (s ~ 5e-4 makes var comparable to
  EPS), so stats and thresholds are corrected by rho = s/s_hat per channel.
- sign ops: ACT Sign(scale, bias) for most images; a 2-op DVE threshold
  (is_ge + affine to fp8) for some, to balance engine load.
- BN batch stats: bn_stats/bn_aggr per core, AllGather of [C,2] partial
  moments, single strided gather DMA, on-chip reduce, Newton-refined rsqrt.
"""

import numpy as np
import ml_dtypes

import concourse.bass as bass
import concourse.mybir as mybir
import concourse.tile as tile
from concourse import bacc
from concourse.bass_utils import run_bass_kernel_spmd

F32 = mybir.dt.float32
F16 = mybir.dt.float16
FP8 = mybir.dt.float8e4
PITCH = 64
AF = mybir.ActivationFunctionType
OP = mybir.AluOpType

N_CORES = 8
N_LOC = 8          # images per core
C = 128            # channels (== partitions)
H = W = 56
HW = H * W         # 3136
EPS = 1e-5
PROWS = H + 4      # padded rows + 2 spare (garbage-col / zero-pair overreads)
TILE_ROWS = 7
N_TILES = H // TILE_ROWS     # 8
QSPAN = TILE_ROWS * PITCH    # 448 padded cols per tile
GTILES = 4                   # tiles per PSUM group (4 banks)
N_GROUPS = N_TILES // GTILES # 2
GCOMPACT = GTILES * TILE_ROWS * W   # 1568 compact elems per group
SCHUNK = TILE_ROWS * W       # 392, bn_stats chunk (4 per group)

# flat padded offsets of the 9 taps (dh*PITCH + dw), natural order; pairs are
# (0,1) (2,3) (4,5) (6,7), single tap 8 is paired with zero weights (slot 9)
TAP_OFF = [0, 1, 2, PITCH, PITCH + 1, PITCH + 2, 2 * PITCH, 2 * PITCH + 1,
           2 * PITCH + 2]
PAIR_D = [TAP_OFF[1] - TAP_OFF[0], TAP_OFF[3] - TAP_OFF[2],
          TAP_OFF[5] - TAP_OFF[4], TAP_OFF[7] - TAP_OFF[6], 1]

# pp param columns. S1H/S2H: power-of-2 rounding of the conv scales s=mean|w|
# (exact in f16); R1/R2: residual ratio rho = s/s_hat, applied to stats and
# sign thresholds so BN's var+EPS sees the reference-scaled values.
(P_G1, P_B1, P_G2, P_B2, P_G3, P_B3, P_A1, P_A2, P_A3,
 P_S1H, P_R1, P_S2H, P_R2,
 P_NG1, P_GR2, P_NGR2, P_GR3, P_NGR3) = range(18)
NP = 18

DR = mybir.MatmulPerfMode.DoubleRow


def _col(t, j):
    return t[:, j : j + 1]


def _rstd_from_allreduced(nc, pool, ar, name):
    """ar: [128,2] = sum over cores of [mean_i, var_i + mean_i^2].
    Returns (mean, rstd) tiles [128,1] f32 with rstd = 1/sqrt(var+EPS),
    Newton-refined to cover ScalarE Sqrt spline error.
    Chain kept short: +EPS folded into the ex2 scaling (off the mean->vpe
    path), Newton's first two multiplies fused into one stt."""
    mean = pool.tile([C, 1], F32, name=f"mean_{name}", tag=f"mean_{name}")
    ex2e = pool.tile([C, 1], F32, name=f"ex2_{name}", tag="sc_ex2")
    nc.vector.tensor_scalar_mul(mean[:], _col(ar, 0), 1.0 / N_CORES)
    nc.vector.tensor_scalar(ex2e[:], _col(ar, 1), 1.0 / N_CORES, EPS,
                            OP.mult, OP.add)
    negmean = pool.tile([C, 1], F32, name=f"negmean_{name}", tag="sc_negmean")
    nc.vector.tensor_scalar_mul(negmean[:], mean[:], -1.0)
    vpe = pool.tile([C, 1], F32, name=f"vpe_{name}", tag="sc_vpe")
    nc.vector.scalar_tensor_tensor(vpe[:], mean[:], negmean[:], ex2e[:],
                                   OP.mult, OP.add)
    rec = pool.tile([C, 1], F32, name=f"rec_{name}", tag="sc_rec")
    nc.vector.reciprocal(rec[:], vpe[:])
    rstd = pool.tile([C, 1], F32, name=f"rstd_{name}", tag=f"rstd_{name}")
    nc.scalar.activation(rstd[:], rec[:], AF.Sqrt)
    # Newton: y <- y * (1.5 - 0.5 * vpe * y^2)
    t1 = pool.tile([C, 1], F32, name=f"t1_{name}", tag="sc_t1")
    nc.vector.scalar_tensor_tensor(t1[:], rstd[:], rstd[:], vpe[:],
                                   OP.mult, OP.mult)
    nc.vector.tensor_scalar(t1[:], t1[:], -0.5, 1.5, OP.mult, OP.add)
    nc.vector.tensor_tensor(out=rstd[:], in0=rstd[:], in1=t1[:], op=OP.mult)
    return mean, rstd


def _affine_consts(nc, pool, pp, mean, rstd, gr_col, ngr_col, b_col, name,
                   rho=None):
    """ks = (g*rho) * rstd; cb = b - (mean/rho) * ks = b - mean*g*rstd.
    gr_col/ngr_col are precomputed g*rho and -g*rho columns so k and -k
    derive from rstd in parallel (shortens the post-collective chain).
    tau = -cb/ks is the threshold in consumer-input units (ks > 0)."""
    k = pool.tile([C, 1], F32, name=f"k_{name}", tag=f"k_{name}")
    nc.vector.tensor_tensor(out=k[:], in0=gr_col, in1=rstd[:], op=OP.mult)
    negk = pool.tile([C, 1], F32, name=f"negk_{name}", tag="sc_negk")
    nc.vector.tensor_tensor(out=negk[:], in0=ngr_col, in1=rstd[:], op=OP.mult)
    cb = pool.tile([C, 1], F32, name=f"cb_{name}", tag=f"cb_{name}")
    if rho is None:
        nc.vector.scalar_tensor_tensor(
            cb[:], mean[:], negk[:], _col(pp, b_col), OP.mult, OP.add
        )
    else:
        # mean is reference-unit; consumer-unit mean is mean/rho (mc is
        # computable before rstd arrives, so it's off the critical path)
        mc = pool.tile([C, 1], F32, name=f"mc_{name}", tag="sc_mc")
        rrho = pool.tile([C, 1], F32, name=f"rrho_{name}", tag="sc_rrho")
        nc.vector.reciprocal(rrho[:], rho)
        nc.vector.tensor_tensor(out=mc[:], in0=mean[:], in1=rrho[:], op=OP.mult)
        nc.vector.scalar_tensor_tensor(
            cb[:], mc[:], negk[:], _col(pp, b_col), OP.mult, OP.add
        )
    # tau = -cb/ks (threshold in consumer-input units)
    rk = pool.tile([C, 1], F32, name=f"rk_{name}", tag="sc_rk")
    nc.vector.reciprocal(rk[:], k[:])
    tau = pool.tile([C, 1], F32, name=f"tau_{name}", tag=f"tau_{name}")
    nc.vector.tensor_tensor(out=tau[:], in0=cb[:], in1=rk[:], op=OP.mult)
    nc.vector.tensor_scalar_mul(tau[:], tau[:], -1.0)
    return k, cb, tau


import os

CONV_MODE = os.environ.get("K_CONV_MODE", "dr6z")  # dr6z | dr3
GATHER1 = os.environ.get("K_GATHER1", "1") == "1"  # single gather DMA
DVE_SIGN = os.environ.get("K_DVE_SIGN", "1") == "1"


def build_nc(reps=1, dbg=False):
    nc = bacc.Bacc(None, target_bir_lowering=False, debug=False, num_devices=N_CORES)

    x_d = nc.dram_tensor("x", [N_LOC, C, HW], F32, kind="ExternalInput")
    w1_d = nc.dram_tensor("w1t", [C, 10, C], FP8, kind="ExternalInput")
    w2_d = nc.dram_tensor("w2t", [C, 10, C], FP8, kind="ExternalInput")
    pp_d = nc.dram_tensor("pp", [C, NP], F32, kind="ExternalInput")
    out_d = nc.dram_tensor("out", [N_LOC, C, HW], F16, kind="ExternalOutput")
    if dbg:
        dbg_pad_d = nc.dram_tensor("dbg_pad", [C, PROWS, PITCH], FP8,
                                   kind="ExternalOutput")
        dbg_c1_d = nc.dram_tensor("dbg_c1", [C, N_LOC, HW], F16,
                                  kind="ExternalOutput")
        dbg_c2_d = nc.dram_tensor("dbg_c2", [C, N_LOC, HW], F16,
                                  kind="ExternalOutput")
        dbg_k_d = nc.dram_tensor("dbg_k", [C, 8], F32, kind="ExternalOutput")

    with tile.TileContext(nc) as tc:
        with (
            tc.tile_pool(name="const", bufs=1) as const,
            tc.tile_pool(name="work", bufs=2) as work,
            tc.tile_pool(name="psum", bufs=2, space="PSUM") as psum,
            tc.tile_pool(name="dram", bufs=1, space="DRAM") as dram,
        ):
            # ---- persistent SBUF tensors ----
            pp = const.tile([C, NP], F32)
            nc.gpsimd.dma_start(pp[:], pp_d[:])
            w1s = const.tile([C, 10, C], FP8)
            w2s = const.tile([C, 10, C], FP8)
            nc.gpsimd.dma_start(w1s[:], w1_d[:])
            nc.gpsimd.dma_start(w2s[:], w2_d[:])
            xb = const.tile([C, N_LOC, HW], F16)    # f16 copy of x
            c1f = const.tile([C, N_LOC, HW], F16)   # p1 = prelu_a1(c1)
            c2f = const.tile([C, N_LOC, HW], F16)   # p2 = prelu_a2(c2)
            stats1 = const.tile([C, N_LOC * 8, 6], F32, tag="stats1")
            stats2 = const.tile([C, N_LOC * 8, 6], F32, tag="stats2")
            stats3 = const.tile([C, N_LOC * 8, 6], F32, tag="stats3")
            N_PADS = 3
            pads = []
            for j in range(N_PADS):
                p = const.tile([C, PROWS, PITCH], FP8, name=f"pad{j}")
                nc.gpsimd.memset(p[:], 0.0)
                pads.append(p)

            a1 = _col(pp, P_A1)
            a2 = _col(pp, P_A2)
            a3 = _col(pp, P_A3)
            s1h = _col(pp, P_S1H)
            s2h = _col(pp, P_S2H)
            r1 = _col(pp, P_R1)
            r2 = _col(pp, P_R2)
            r1sq = const.tile([C, 1], F32, name="r1sq")
            nc.vector.tensor_tensor(out=r1sq[:], in0=r1, in1=r1, op=OP.mult)
            r2sq = const.tile([C, 1], F32, name="r2sq")
            nc.vector.tensor_tensor(out=r2sq[:], in0=r2, in1=r2, op=OP.mult)

            cc_counter = [0]

            def reduce_stats(stats, idx, rho=None, rhosq=None):
                """bn_aggr + pack [mean, var+mean^2] (rescaled into reference
                units by rho) + allgather-sum. Returns [128,2] tile of global
                [sum mean_i, sum (var_i+m_i^2)]."""
                mv = const.tile([C, 2], F32, name=f"mv{idx}", tag="sc_mv")
                nc.vector.bn_aggr(mv[:], stats[:])
                e = const.tile([C, 2], F32, name=f"e{idx}", tag="sc_e")
                if rho is None:
                    nc.vector.tensor_copy(_col(e, 0), _col(mv, 0))
                else:
                    nc.vector.tensor_tensor(out=_col(e, 0), in0=_col(mv, 0),
                                            in1=rho, op=OP.mult)
                nc.vector.scalar_tensor_tensor(
                    _col(e, 1), _col(mv, 0), _col(mv, 0), _col(mv, 1), OP.mult, OP.add
                )
                if rhosq is not None:
                    nc.vector.tensor_tensor(out=_col(e, 1), in0=_col(e, 1),
                                            in1=rhosq, op=OP.mult)
                n = cc_counter[0]
                cc_counter[0] += 1
                cci = dram.tile([C, 2], F32, name=f"cc_in{n}", tag=f"cc_in{n}")
                cco = dram.tile([N_CORES, C, 2], F32, name=f"cc_out{n}",
                                tag=f"cc_out{n}", addr_space="Shared")
                nc.sync.dma_start(cci[:], e[:])
                nc.gpsimd.collective_compute(
                    "AllGather",
                    OP.bypass,
                    replica_groups=[list(range(N_CORES))],
                    ins=[cci.opt()],
                    outs=[cco.opt()],
                )
                # gather: g8[c, j, r] <- cco[r, c, j]
                g8 = const.tile([C, 2, N_CORES], F32, name=f"g8{idx}", tag="sc_g8")
                if GATHER1:
                    c0 = cco[0]
                    in_ap = bass.AP(c0.tensor, c0.offset,
                                    [list(c0.ap[0]), [1, 2], [2 * C, N_CORES]])
                    nc.sync.dma_start(g8[:], in_ap)
                else:
                    for r in range(N_CORES):
                        nc.sync.dma_start(g8[:, :, r], cco[r])
                g = const.tile([C, 2], F32, name=f"g{idx}", tag="sc_g")
                nc.vector.tensor_reduce(g[:], g8[:], mybir.AxisListType.X, OP.add)
                return g

            def conv(pad, ws, dst, stats, i, acol, shcol):
                """3x3 conv of padded +/-1 fp8 image (row pitch 64) -> PReLU'd
                f16 dst [C,HW]. 5 fp8 DoubleRow passes per 448-col tile (pass 4
                pairs the last tap with zero weights). Evacuation fuses PReLU;
                engine alternates ACT/DVE for load balance; bn_stats chunks
                follow each group."""
                padf = pad[:].rearrange("p r w -> p (r w)")
                for g in range(N_GROUPS):
                    tiles = range(g * GTILES, (g + 1) * GTILES)
                    psg = psum.tile([C, GTILES, 512], F32, tag="ps",
                                    name=f"ps{g}", bufs=2)
                    if CONV_MODE == "dr6z":
                        # 6 DoubleRow passes, all rhs pair strides = PITCH
                        # (16B-aligned, HW requirement): 3 vertical tap pairs
                        # (dh 0+1) + 3 (dh=2 tap, zero-slot-9) pairs.
                        wb = ws[:, 0, :]
                        for p_ in range(6):
                            if p_ < 3:
                                woff, wstride, base = p_ * C, 3 * C, TAP_OFF[p_]
                            else:
                                woff = (3 + p_) * C      # slots 6,7,8
                                wstride = (9 - (3 + p_)) * C  # to zero slot 9
                                base = TAP_OFF[3 + p_]
                            wp = bass.AP(wb.tensor, wb.offset + woff,
                                         [list(wb.ap[0]), [wstride, 2], [1, C]])
                            for j, t in enumerate(tiles):
                                q0 = t * QSPAN + base
                                rhs = bass.AP(padf.tensor, padf.offset + q0,
                                              [list(padf.ap[0]), [PITCH, 2],
                                               [1, QSPAN]])
                                nc.tensor.matmul(
                                    psg[:, j, 0:QSPAN], wp, rhs,
                                    start=(p_ == 0), stop=(p_ == 5),
                                    perf_mode=DR,
                                )
                    else:
                        # dr3: vertical pairs (0,dw)+(1,dw) with rhs pair
                        # stride PITCH, then 3 plain passes for dh=2 taps.
                        wb = ws[:, 0, :]
                        for dw in range(3):
                            wp = bass.AP(wb.tensor, wb.offset + dw * C,
                                         [list(wb.ap[0]), [3 * C, 2], [1, C]])
                            for j, t in enumerate(tiles):
                                q0 = t * QSPAN + dw
                                rhs = bass.AP(padf.tensor, padf.offset + q0,
                                              [list(padf.ap[0]), [PITCH, 2],
                                               [1, QSPAN]])
                                nc.tensor.matmul(
                                    psg[:, j, 0:QSPAN], wp, rhs,
                                    start=(dw == 0), stop=False,
                                    perf_mode=DR,
                                )
                        for dw in range(3):
                            for j, t in enumerate(tiles):
                                q0 = t * QSPAN + 2 * PITCH + dw
                                nc.tensor.matmul(
                                    psg[:, j, 0:QSPAN], ws[:, 6 + dw, :],
                                    padf[:, q0 : q0 + QSPAN],
                                    start=False, stop=(dw == 2),
                                )
                    gbase = psg[:]
                    # For the batch's last image, split the final group's
                    # evacuation in half so its bn_stats (which gate the
                    # collective) start one half earlier.
                    nsplit = 2 if (i == N_LOC - 1 and g == N_GROUPS - 1) else 1
                    tper = GTILES // nsplit
                    for h in range(nsplit):
                        src_ap = bass.AP(gbase.tensor,
                                         gbase.offset + h * tper * 512,
                                         [list(gbase.ap[0]), [512, tper],
                                          [PITCH, TILE_ROWS], [1, W]])
                        off = g * GCOMPACT + h * tper * SCHUNK
                        dst_sl = dst[:, off : off + tper * SCHUNK]
                        dst_ap = dst_sl.rearrange("p (t r w) -> p t r w",
                                                  t=tper, r=TILE_ROWS, w=W)
                        nc.scalar.activation(dst_ap, src_ap, AF.Prelu,
                                             alpha=acol, scale=shcol)
                        for k in range(tper):
                            kk = h * tper + k
                            nc.vector.bn_stats(
                                stats[:, i * 8 + g * GTILES + kk, :],
                                dst[:, g * GCOMPACT + kk * SCHUNK
                                       : g * GCOMPACT + (kk + 1) * SCHUNK])

            for _rep in range(reps):
                # ============ Phase A: load x, convert f16, BN1 stats ========
                # Last image loads in quarters so its final bn_stats (which
                # gate the AR1 collective) start a quarter-chunk earlier.
                for i in range(N_LOC):
                    ndma = 1
                    for h in range(2):
                        xin = work.tile([C, HW // 2], F32, tag="xin", bufs=3,
                                        name=f"xa{i}_{h}")
                        qsz = (HW // 2) // ndma
                        for q in range(ndma):
                            nc.sync.dma_start(
                                xin[:, q * qsz : (q + 1) * qsz],
                                x_d[i, :, h * (HW // 2) + q * qsz
                                      : h * (HW // 2) + (q + 1) * qsz])
                        xbsl = xb[:, i, h * (HW // 2) : (h + 1) * (HW // 2)]
                        nc.scalar.activation(xbsl, xin[:], AF.Copy)
                        for k in range(4):
                            # stats from the f32 data: the f16 copy shifts the
                            # BN1 mean enough (~1e-6) to flip near-threshold
                            # signs, which cascades through both binary convs
                            nc.vector.bn_stats(
                                stats1[:, i * 8 + h * 4 + k, :],
                                xin[:, k * SCHUNK : (k + 1) * SCHUNK])

                g1ar = reduce_stats(stats1, 0)
                mean1, rstd1 = _rstd_from_allreduced(nc, const, g1ar, "1")
                k1, c1b, tau1 = _affine_consts(
                    nc, const, pp, mean1, rstd1,
                    _col(pp, P_G1), _col(pp, P_NG1), P_B1, "1")

                def sign_to_pad(i, src_img, k, cb, tau):
                    """pad interior <- sign(k*src + cb) as +/-1 fp8.
                    DVE (2-op threshold) for some images, ACT for the rest.
                    High priority: the sign gates the PE for this image."""
                    pad = pads[i % N_PADS]
                    dst = pad[:, 1 : H + 1, 1 : W + 1]
                    with tc.high_priority(offset=60):
                        if DVE_SIGN and i in (1, 4, 6):
                            t01 = work.tile([C, HW], F16, tag="d1", bufs=3,
                                            name=f"t01_{i}")
                            nc.vector.tensor_scalar(t01[:], src_img, tau[:],
                                                    None, OP.is_ge)
                            nc.vector.tensor_scalar(
                                dst,
                                t01[:].rearrange("p (h w) -> p h w", h=H, w=W),
                                2.0, -1.0, OP.mult, OP.add)
                        else:
                            nc.scalar.activation(
                                dst,
                                src_img.rearrange("p (h w) -> p h w", h=H, w=W),
                                AF.Sign, bias=cb[:], scale=k[:],
                            )
                    return pad

                # ============ Phase B: b1 = sign(BN1(x)); conv1; stats2 ======
                # sign1 thresholds x near tau1 and errors cascade through two
                # binary convs, so it must read x at full f32 precision:
                # re-load x from HBM (prefetches during phase A / AR1).
                HROWS = H // 2  # 28
                for i in range(N_LOC):
                    pad = pads[i % N_PADS]
                    for h in range(2):
                        with tc.high_priority(offset=60):
                            xin = work.tile([C, HW // 2], F32, tag="xin",
                                            bufs=3, name=f"xs{i}_{h}")
                            nc.sync.dma_start(
                                xin[:],
                                x_d[i, :, h * (HW // 2) : (h + 1) * (HW // 2)])
                            dsth = pad[:, 1 + h * HROWS : 1 + (h + 1) * HROWS,
                                       1 : W + 1]
                            if DVE_SIGN and i in (1, 4, 6):
                                t01 = work.tile([C, HW], F16, tag="d1",
                                                bufs=3, name=f"t01_{i}_{h}")
                                t01h = t01[:, 0 : HW // 2]
                                nc.vector.tensor_scalar(t01h, xin[:], tau1[:],
                                                        None, OP.is_ge)
                                nc.vector.tensor_scalar(
                                    dsth,
                                    t01h.rearrange("p (h w) -> p h w",
                                                   h=HROWS, w=W),
                                    2.0, -1.0, OP.mult, OP.add)
                            else:
                                nc.scalar.activation(
                                    dsth,
                                    xin[:].rearrange("p (h w) -> p h w",
                                                     h=HROWS, w=W),
                                    AF.Sign, bias=c1b[:], scale=k1[:],
                                )
                    if dbg and i == 0:
                        nc.sync.dma_start(dbg_pad_d[:], pad[:])
                    conv(pad, w1s, c1f[:, i, :], stats2, i, a1, s1h)

                g2ar = reduce_stats(stats2, 1, r1, r1sq[:])
                mean2, rstd2 = _rstd_from_allreduced(nc, const, g2ar, "2")
                k2, c2b, tau2 = _affine_consts(
                    nc, const, pp, mean2, rstd2,
                    _col(pp, P_GR2), _col(pp, P_NGR2), P_B2, "2", rho=r1)

                # ============ Phase C: b2 = sign(BN2(p1)); conv2; stats3 =====
                for i in range(N_LOC):
                    pad = sign_to_pad(i, c1f[:, i, :], k2, c2b, tau2)
                    conv(pad, w2s, c2f[:, i, :], stats3, i, a2, s2h)

                g3ar = reduce_stats(stats3, 2, r2, r2sq[:])
                mean3, rstd3 = _rstd_from_allreduced(nc, const, g3ar, "3")
                k3, c3b, _tau3 = _affine_consts(
                    nc, const, pp, mean3, rstd3,
                    _col(pp, P_GR3), _col(pp, P_NGR3), P_B3, "3", rho=r2)

                if dbg:
                    nc.sync.dma_start(dbg_c1_d[:], c1f[:])
                    nc.sync.dma_start(dbg_c2_d[:], c2f[:])
                    dbgk = const.tile([C, 8], F32)
                    for j, t_ in enumerate(
                        [k1, c1b, tau1, k2, c2b, tau2, k3, c3b]
                    ):
                        nc.vector.tensor_copy(_col(dbgk, j), t_[:])
                    nc.sync.dma_start(dbg_k_d[:], dbgk[:])

                # ====== Phase D: y = PReLU(k3*p2 + x + c3b) ======
                for i in range(N_LOC):
                    d1 = work.tile([C, HW], F16, tag="d1", bufs=3)
                    nc.vector.tensor_scalar(d1[:], c2f[:, i, :], k3[:],
                                            None, OP.mult)
                    nc.vector.tensor_tensor(out=d1[:], in0=d1[:],
                                            in1=xb[:, i, :], op=OP.add)
                    if i >= N_LOC - 2:
                        # halve the trailing prelu+DMA chain of the last images
                        for h in range(2):
                            sl = slice(h * (HW // 2), (h + 1) * (HW // 2))
                            nc.scalar.activation(c1f[:, i, sl], d1[:, sl],
                                                 AF.Prelu, bias=c3b[:],
                                                 alpha=a3)
                            nc.sync.dma_start(out_d[i][:, sl], c1f[:, i, sl])
                    else:
                        nc.scalar.activation(c1f[:, i, :], d1[:], AF.Prelu,
                                             bias=c3b[:], alpha=a3)
                        nc.sync.dma_start(out_d[i], c1f[:, i, :])

    nc.compile()
    return nc


def _prep_host(x, bn1_g, bn1_b, w1, prelu1_a, bn2_g, bn2_b, w2, prelu2_a,
               bn3_g, bn3_b, prelu3_a):
    def wprep(w_flat):
        w = np.asarray(w_flat, np.float32).reshape(C, C, 3, 3)
        # lhsT layout [i, slot, o] = sign(w[o, i, dh, dw]); slot 9 zero-padded
        wT = np.sign(w).transpose(1, 2, 3, 0).reshape(C, 9, C)
        w10 = np.zeros((C, 10, C), np.float32)
        w10[:, :9, :] = wT
        s = np.mean(np.abs(w), axis=(1, 2, 3)).astype(np.float32)  # [C] per o
        s_hat = np.exp2(np.round(np.log2(s))).astype(np.float32)
        rho = (s / s_hat).astype(np.float32)
        return w10.astype(mybir.dt.np(FP8)), s_hat, rho

    w1t, s1h_, r1_ = wprep(w1)
    w2t, s2h_, r2_ = wprep(w2)

    pp = np.zeros((C, NP), np.float32)
    pp[:, P_S1H] = s1h_
    pp[:, P_R1] = r1_
    pp[:, P_S2H] = s2h_
    pp[:, P_R2] = r2_
    g1_ = np.asarray(bn1_g, np.float32)
    g2_ = np.asarray(bn2_g, np.float32)
    g3_ = np.asarray(bn3_g, np.float32)
    pp[:, P_NG1] = -g1_
    pp[:, P_GR2] = g2_ * r1_
    pp[:, P_NGR2] = -(g2_ * r1_)
    pp[:, P_GR3] = g3_ * r2_
    pp[:, P_NGR3] = -(g3_ * r2_)
    pp[:, P_G1] = np.asarray(bn1_g, np.float32)
    pp[:, P_B1] = np.asarray(bn1_b, np.float32)
    pp[:, P_G2] = np.asarray(bn2_g, np.float32)
    pp[:, P_B2] = np.asarray(bn2_b, np.float32)
    pp[:, P_G3] = np.asarray(bn3_g, np.float32)
    pp[:, P_B3] = np.asarray(bn3_b, np.float32)
    pp[:, P_A1] = np.float32(prelu1_a)
    pp[:, P_A2] = np.float32(prelu2_a)
    pp[:, P_A3] = np.float32(prelu3_a)

    x = np.ascontiguousarray(np.asarray(x, np.float32).reshape(64, C, HW))
    in_maps = []
    for r in range(N_CORES):
        in_maps.append({
            "x": x[r * N_LOC : (r + 1) * N_LOC],
            "w1t": w1t,
            "w2t": w2t,
            "pp": pp,
        })
    return in_maps


_NC_CACHE = None


def _get_nc():
    global _NC_CACHE
    if _NC_CACHE is None:
        _NC_CACHE = build_nc()
    return _NC_CACHE


def run(in_maps, **kwargs):
    nc = _get_nc()
    return run_bass_kernel_spmd(nc, in_maps, core_ids=list(range(N_CORES)), **kwargs)


def kernel(**inputs):
    in_maps = _prep_host(**inputs)
    last_err = None
    for attempt in range(3):
        try:
            res = run(in_maps)
            break
        except Exception as e:  # transient NRT device errors happen; retry
            last_err = e
            import time as _time
            _time.sleep(2.0)
    else:
        raise last_err
    out = np.concatenate(
        [np.asarray(r["out"]).astype(np.float32).reshape(N_LOC, C, H, W)
         for r in res.results], axis=0
    )
    return out


if __name__ == "__main__":
    rng = np.random.default_rng(0)
    x = rng.standard_normal((64, C, H, W), dtype=np.float32)
    w1 = ((rng.random((C * C * 9, 1), dtype=np.float32) - 0.5) * 0.002)
    w2 = ((rng.random((C * C * 9, 1), dtype=np.float32) - 0.5) * 0.002)
    ones = np.ones(C, np.float32)
    zeros = np.zeros(C, np.float32)
    y = kernel(x=x, bn1_g=ones, bn1_b=zeros, w1=w1, prelu1_a=np.float32(0.25),
               bn2_g=ones, bn2_b=zeros, w2=w2, prelu2_a=np.float32(0.25),
               bn3_g=ones, bn3_b=zeros, prelu3_a=np.float32(0.25))
    print("out", y.shape, y.dtype, float(np.abs(y).mean()))


# revision 50
# speedup vs baseline: 1.0058x; 1.0020x over previous
"""XNOR-Net BasicBlock forward (BN-sign-binconv-PReLU x2 + BN + residual + PReLU)
distributed over 8 Trainium2 NeuronCores, data-parallel over the batch axis.

Self-contained: hardcodes shapes N=64, C=128, H=W=56, 8 cores.

Design notes:
- Phase A: x streams from HBM (f32) feeding BN1 bn_stats (full f32 precision:
  the sign1 threshold cascades through both binary convs, so stats from a
  rounded copy flip near-threshold pixels) and an f16 copy (xb) kept in SBUF
  for the phase-D residual. Phase B re-loads x (f32) for the sign itself.
- 3x3 binary conv: 6 fp8 DoubleRow matmul passes per 448-col PSUM tile — 3
  vertical tap pairs (dh 0+1) plus 3 (dh=2 tap, zero-weight slot) pairs. All
  rhs pair strides = PITCH = 64 (DoubleRow requires 16-aligned pair strides
  on HW; unaligned strides wedge the PE). 3 PE cycles/pixel.
- PSUM evacuation is a fused ACT Prelu with scale = s_hat (the power-of-2
  rounding of the conv scale s = mean|w|, so p' = s_hat*prelu(c) is exact in
  f16). BN's var+EPS is NOT scale-invariant (s ~ 5e-4 makes var comparable to
  EPS), so stats and thresholds are corrected by rho = s/s_hat per channel.
- sign ops: ACT Sign(scale, bias) for most images; a 2-op DVE threshold
  (is_ge + affine to fp8) for some, to balance engine load.
- BN batch stats: bn_stats/bn_aggr per core, AllGather of [C,2] partial
  moments, single strided gather DMA, on-chip reduce, Newton-refined rsqrt.
"""

import numpy as np
import ml_dtypes

import concourse.bass as bass
import concourse.mybir as mybir
import concourse.tile as tile
from concourse import bacc
from concourse.bass_utils import run_bass_kernel_spmd

F32 = mybir.dt.float32
F16 = mybir.dt.float16
FP8 = mybir.dt.float8e4
PITCH = 64
AF = mybir.ActivationFunctionType
OP = mybir.AluOpType

N_CORES = 8
N_LOC = 8          # images per core
C = 128            # channels (== partitions)
H = W = 56
HW = H * W         # 3136
EPS = 1e-5
PROWS = H + 4      # padded rows + 2 spare (garbage-col / zero-pair overreads)
TILE_ROWS = 7
N_TILES = H // TILE_ROWS     # 8
QSPAN = TILE_ROWS * PITCH    # 448 padded cols per tile
GTILES = 4                   # tiles per PSUM group (4 banks)
N_GROUPS = N_TILES // GTILES # 2
GCOMPACT = GTILES * TILE_ROWS * W   # 1568 compact elems per group
SCHUNK = TILE_ROWS * W       # 392, bn_stats chunk (4 per group)

# flat padded offsets of the 9 taps (dh*PITCH + dw), natural order; pairs are
# (0,1) (2,3) (4,5) (6,7), single tap 8 is paired with zero weights (slot 9)
TAP_OFF = [0, 1, 2, PITCH, PITCH + 1, PITCH + 2, 2 * PITCH, 2 * PITCH + 1,
           2 * PITCH + 2]
PAIR_D = [TAP_OFF[1] - TAP_OFF[0], TAP_OFF[3] - TAP_OFF[2],
          TAP_OFF[5] - TAP_OFF[4], TAP_OFF[7] - TAP_OFF[6], 1]

# pp param columns. S1H/S2H: power-of-2 rounding of the conv scales s=mean|w|
# (exact in f16); R1/R2: residual ratio rho = s/s_hat, applied to stats and
# sign thresholds so BN's var+EPS sees the reference-scaled values.
(P_G1, P_B1, P_G2, P_B2, P_G3, P_B3, P_A1, P_A2, P_A3,
 P_S1H, P_R1, P_S2H, P_R2,
 P_NG1, P_GR2, P_NGR2, P_GR3, P_NGR3) = range(18)
NP = 18

DR = mybir.MatmulPerfMode.DoubleRow


def _col(t, j):
    return t[:, j : j + 1]


def _rstd_from_allreduced(nc, pool, ar, name):
    """ar: [128,2] = sum over cores of [mean_i, var_i + mean_i^2].
    Returns (mean, rstd) tiles [128,1] f32 with rstd = 1/sqrt(var+EPS),
    Newton-refined to cover ScalarE Sqrt spline error.
    Chain kept short: +EPS folded into the ex2 scaling (off the mean->vpe
    path), Newton's first two multiplies fused into one stt."""
    mean = pool.tile([C, 1], F32, name=f"mean_{name}", tag=f"mean_{name}")
    ex2e = pool.tile([C, 1], F32, name=f"ex2_{name}", tag="sc_ex2")
    nc.vector.tensor_scalar_mul(mean[:], _col(ar, 0), 1.0 / N_CORES)
    nc.vector.tensor_scalar(ex2e[:], _col(ar, 1), 1.0 / N_CORES, EPS,
                            OP.mult, OP.add)
    negmean = pool.tile([C, 1], F32, name=f"negmean_{name}", tag="sc_negmean")
    nc.vector.tensor_scalar_mul(negmean[:], mean[:], -1.0)
    vpe = pool.tile([C, 1], F32, name=f"vpe_{name}", tag="sc_vpe")
    nc.vector.scalar_tensor_tensor(vpe[:], mean[:], negmean[:], ex2e[:],
                                   OP.mult, OP.add)
    rec = pool.tile([C, 1], F32, name=f"rec_{name}", tag="sc_rec")
    nc.vector.reciprocal(rec[:], vpe[:])
    rstd = pool.tile([C, 1], F32, name=f"rstd_{name}", tag=f"rstd_{name}")
    nc.scalar.activation(rstd[:], rec[:], AF.Sqrt)
    # Newton: y <- y * (1.5 - 0.5 * vpe * y^2)
    t1 = pool.tile([C, 1], F32, name=f"t1_{name}", tag="sc_t1")
    nc.vector.scalar_tensor_tensor(t1[:], rstd[:], rstd[:], vpe[:],
                                   OP.mult, OP.mult)
    nc.vector.tensor_scalar(t1[:], t1[:], -0.5, 1.5, OP.mult, OP.add)
    nc.vector.tensor_tensor(out=rstd[:], in0=rstd[:], in1=t1[:], op=OP.mult)
    return mean, rstd


def _affine_consts(nc, pool, pp, mean, rstd, gr_col, ngr_col, b_col, name,
                   rho=None):
    """ks = (g*rho) * rstd; cb = b - (mean/rho) * ks = b - mean*g*rstd.
    gr_col/ngr_col are precomputed g*rho and -g*rho columns so k and -k
    derive from rstd in parallel (shortens the post-collective chain).
    tau = -cb/ks is the threshold in consumer-input units (ks > 0)."""
    k = pool.tile([C, 1], F32, name=f"k_{name}", tag=f"k_{name}")
    nc.vector.tensor_tensor(out=k[:], in0=gr_col, in1=rstd[:], op=OP.mult)
    negk = pool.tile([C, 1], F32, name=f"negk_{name}", tag="sc_negk")
    nc.vector.tensor_tensor(out=negk[:], in0=ngr_col, in1=rstd[:], op=OP.mult)
    cb = pool.tile([C, 1], F32, name=f"cb_{name}", tag=f"cb_{name}")
    if rho is None:
        nc.vector.scalar_tensor_tensor(
            cb[:], mean[:], negk[:], _col(pp, b_col), OP.mult, OP.add
        )
    else:
        # mean is reference-unit; consumer-unit mean is mean/rho (mc is
        # computable before rstd arrives, so it's off the critical path)
        mc = pool.tile([C, 1], F32, name=f"mc_{name}", tag="sc_mc")
        rrho = pool.tile([C, 1], F32, name=f"rrho_{name}", tag="sc_rrho")
        nc.vector.reciprocal(rrho[:], rho)
        nc.vector.tensor_tensor(out=mc[:], in0=mean[:], in1=rrho[:], op=OP.mult)
        nc.vector.scalar_tensor_tensor(
            cb[:], mc[:], negk[:], _col(pp, b_col), OP.mult, OP.add
        )
    # tau = -cb/ks (threshold in consumer-input units)
    rk = pool.tile([C, 1], F32, name=f"rk_{name}", tag="sc_rk")
    nc.vector.reciprocal(rk[:], k[:])
    tau = pool.tile([C, 1], F32, name=f"tau_{name}", tag=f"tau_{name}")
    nc.vector.tensor_tensor(out=tau[:], in0=cb[:], in1=rk[:], op=OP.mult)
    nc.vector.tensor_scalar_mul(tau[:], tau[:], -1.0)
    return k, cb, tau


import os

CONV_MODE = os.environ.get("K_CONV_MODE", "dr6z")  # dr6z | dr3
GATHER1 = os.environ.get("K_GATHER1", "1") == "1"  # single gather DMA
DVE_SIGN = os.environ.get("K_DVE_SIGN", "1") == "1"


def build_nc(reps=1, dbg=False):
    nc = bacc.Bacc(None, target_bir_lowering=False, debug=False, num_devices=N_CORES)

    x_d = nc.dram_tensor("x", [N_LOC, C, HW], F32, kind="ExternalInput")
    w1_d = nc.dram_tensor("w1t", [C, 10, C], FP8, kind="ExternalInput")
    w2_d = nc.dram_tensor("w2t", [C, 10, C], FP8, kind="ExternalInput")
    pp_d = nc.dram_tensor("pp", [C, NP], F32, kind="ExternalInput")
    out_d = nc.dram_tensor("out", [N_LOC, C, HW], F16, kind="ExternalOutput")
    if dbg:
        dbg_pad_d = nc.dram_tensor("dbg_pad", [C, PROWS, PITCH], FP8,
                                   kind="ExternalOutput")
        dbg_c1_d = nc.dram_tensor("dbg_c1", [C, N_LOC, HW], F16,
                                  kind="ExternalOutput")
        dbg_c2_d = nc.dram_tensor("dbg_c2", [C, N_LOC, HW], F16,
                                  kind="ExternalOutput")
        dbg_k_d = nc.dram_tensor("dbg_k", [C, 8], F32, kind="ExternalOutput")

    with tile.TileContext(nc) as tc:
        with (
            tc.tile_pool(name="const", bufs=1) as const,
            tc.tile_pool(name="work", bufs=2) as work,
            tc.tile_pool(name="psum", bufs=2, space="PSUM") as psum,
            tc.tile_pool(name="dram", bufs=1, space="DRAM") as dram,
        ):
            # ---- persistent SBUF tensors ----
            pp = const.tile([C, NP], F32)
            nc.gpsimd.dma_start(pp[:], pp_d[:])
            w1s = const.tile([C, 10, C], FP8)
            w2s = const.tile([C, 10, C], FP8)
            nc.gpsimd.dma_start(w1s[:], w1_d[:])
            nc.gpsimd.dma_start(w2s[:], w2_d[:])
            xb = const.tile([C, N_LOC, HW], F16)    # f16 copy of x
            c1f = const.tile([C, N_LOC, HW], F16)   # p1 = prelu_a1(c1)
            c2f = const.tile([C, N_LOC, HW], F16)   # p2 = prelu_a2(c2)
            stats1 = const.tile([C, N_LOC * 8, 6], F32, tag="stats1")
            stats2 = const.tile([C, N_LOC * 8, 6], F32, tag="stats2")
            stats3 = const.tile([C, N_LOC * 8, 6], F32, tag="stats3")
            N_PADS = 3
            pads = []
            for j in range(N_PADS):
                p = const.tile([C, PROWS, PITCH], FP8, name=f"pad{j}")
                nc.gpsimd.memset(p[:], 0.0)
                pads.append(p)

            a1 = _col(pp, P_A1)
            a2 = _col(pp, P_A2)
            a3 = _col(pp, P_A3)
            s1h = _col(pp, P_S1H)
            s2h = _col(pp, P_S2H)
            r1 = _col(pp, P_R1)
            r2 = _col(pp, P_R2)
            r1sq = const.tile([C, 1], F32, name="r1sq")
            nc.vector.tensor_tensor(out=r1sq[:], in0=r1, in1=r1, op=OP.mult)
            r2sq = const.tile([C, 1], F32, name="r2sq")
            nc.vector.tensor_tensor(out=r2sq[:], in0=r2, in1=r2, op=OP.mult)

            cc_counter = [0]

            def reduce_stats(stats, idx, rho=None, rhosq=None):
                """bn_aggr + pack [mean, var+mean^2] (rescaled into reference
                units by rho) + allgather-sum. Returns [128,2] tile of global
                [sum mean_i, sum (var_i+m_i^2)]."""
                mv = const.tile([C, 2], F32, name=f"mv{idx}", tag="sc_mv")
                nc.vector.bn_aggr(mv[:], stats[:])
                e = const.tile([C, 2], F32, name=f"e{idx}", tag="sc_e")
                if rho is None:
                    nc.vector.tensor_copy(_col(e, 0), _col(mv, 0))
                else:
                    nc.vector.tensor_tensor(out=_col(e, 0), in0=_col(mv, 0),
                                            in1=rho, op=OP.mult)
                nc.vector.scalar_tensor_tensor(
                    _col(e, 1), _col(mv, 0), _col(mv, 0), _col(mv, 1), OP.mult, OP.add
                )
                if rhosq is not None:
                    nc.vector.tensor_tensor(out=_col(e, 1), in0=_col(e, 1),
                                            in1=rhosq, op=OP.mult)
                n = cc_counter[0]
                cc_counter[0] += 1
                cci = dram.tile([C, 2], F32, name=f"cc_in{n}", tag=f"cc_in{n}")
                cco = dram.tile([N_CORES, C, 2], F32, name=f"cc_out{n}",
                                tag=f"cc_out{n}", addr_space="Shared")
                nc.sync.dma_start(cci[:], e[:])
                nc.gpsimd.collective_compute(
                    "AllGather",
                    OP.bypass,
                    replica_groups=[list(range(N_CORES))],
                    ins=[cci.opt()],
                    outs=[cco.opt()],
                )
                # gather: g8[c, j, r] <- cco[r, c, j]
                # layout [C, r, j]: the DMA moves contiguous [m,e] pairs
                # (8B elements -> half the descriptors of a j-major layout)
                g8 = const.tile([C, N_CORES, 2], F32, name=f"g8{idx}", tag="sc_g8")
                if GATHER1:
                    c0 = cco[0]
                    in_ap = bass.AP(c0.tensor, c0.offset,
                                    [list(c0.ap[0]), [2 * C, N_CORES], [1, 2]])
                    nc.sync.dma_start(g8[:], in_ap)
                else:
                    for r in range(N_CORES):
                        nc.sync.dma_start(g8[:, r, :], cco[r])
                g = const.tile([C, 2], F32, name=f"g{idx}", tag="sc_g")
                gb = g8[:]
                gview = bass.AP(gb.tensor, gb.offset,
                                [list(gb.ap[0]), [1, 2], [2, N_CORES]])
                nc.vector.tensor_reduce(g[:], gview, mybir.AxisListType.X, OP.add)
                return g

            def conv(pad, ws, dst, stats, i, acol, shcol):
                """3x3 conv of padded +/-1 fp8 image (row pitch 64) -> PReLU'd
                f16 dst [C,HW]. 5 fp8 DoubleRow passes per 448-col tile (pass 4
                pairs the last tap with zero weights). Evacuation fuses PReLU;
                engine alternates ACT/DVE for load balance; bn_stats chunks
                follow each group."""
                padf = pad[:].rearrange("p r w -> p (r w)")
                for g in range(N_GROUPS):
                    tiles = range(g * GTILES, (g + 1) * GTILES)
                    psg = psum.tile([C, GTILES, 512], F32, tag="ps",
                                    name=f"ps{g}", bufs=2)
                    if CONV_MODE == "dr6z":
                        # 6 DoubleRow passes, all rhs pair strides +/-PITCH
                        # (16B-aligned, HW requirement): 3 vertical tap pairs
                        # (dh 0+1) + 3 (dh=2 tap, zero-slot-9) pairs. The
                        # zero pairs read the row ABOVE (stride -PITCH, values
                        # killed by the zero weights) so the first tile group
                        # never reads past its own sign half.
                        wb = ws[:, 0, :]
                        for p_ in range(6):
                            if p_ < 3:
                                woff, wstride, base = p_ * C, 3 * C, TAP_OFF[p_]
                                pstep = PITCH
                            else:
                                woff = (3 + p_) * C      # slots 6,7,8
                                wstride = (9 - (3 + p_)) * C  # to zero slot 9
                                base = TAP_OFF[3 + p_]
                                pstep = -PITCH
                            wp = bass.AP(wb.tensor, wb.offset + woff,
                                         [list(wb.ap[0]), [wstride, 2], [1, C]])
                            for j, t in enumerate(tiles):
                                q0 = t * QSPAN + base
                                rhs = bass.AP(padf.tensor, padf.offset + q0,
                                              [list(padf.ap[0]), [pstep, 2],
                                               [1, QSPAN]])
                                nc.tensor.matmul(
                                    psg[:, j, 0:QSPAN], wp, rhs,
                                    start=(p_ == 0), stop=(p_ == 5),
                                    perf_mode=DR,
                                )
                    else:
                        # dr3: vertical pairs (0,dw)+(1,dw) with rhs pair
                        # stride PITCH, then 3 plain passes for dh=2 taps.
                        wb = ws[:, 0, :]
                        for dw in range(3):
                            wp = bass.AP(wb.tensor, wb.offset + dw * C,
                                         [list(wb.ap[0]), [3 * C, 2], [1, C]])
                            for j, t in enumerate(tiles):
                                q0 = t * QSPAN + dw
                                rhs = bass.AP(padf.tensor, padf.offset + q0,
                                              [list(padf.ap[0]), [PITCH, 2],
                                               [1, QSPAN]])
                                nc.tensor.matmul(
                                    psg[:, j, 0:QSPAN], wp, rhs,
                                    start=(dw == 0), stop=False,
                                    perf_mode=DR,
                                )
                        for dw in range(3):
                            for j, t in enumerate(tiles):
                                q0 = t * QSPAN + 2 * PITCH + dw
                                nc.tensor.matmul(
                                    psg[:, j, 0:QSPAN], ws[:, 6 + dw, :],
                                    padf[:, q0 : q0 + QSPAN],
                                    start=False, stop=(dw == 2),
                                )
                    gbase = psg[:]
                    # For the batch's last image, split the final group's
                    # evacuation in half so its bn_stats (which gate the
                    # collective) start one half earlier.
                    nsplit = 2 if (i == N_LOC - 1 and g == N_GROUPS - 1) else 1
                    tper = GTILES // nsplit
                    for h in range(nsplit):
                        src_ap = bass.AP(gbase.tensor,
                                         gbase.offset + h * tper * 512,
                                         [list(gbase.ap[0]), [512, tper],
                                          [PITCH, TILE_ROWS], [1, W]])
                        off = g * GCOMPACT + h * tper * SCHUNK
                        dst_sl = dst[:, off : off + tper * SCHUNK]
                        dst_ap = dst_sl.rearrange("p (t r w) -> p t r w",
                                                  t=tper, r=TILE_ROWS, w=W)
                        nc.scalar.activation(dst_ap, src_ap, AF.Prelu,
                                             alpha=acol, scale=shcol)
                        for k in range(tper):
                            kk = h * tper + k
                            nc.vector.bn_stats(
                                stats[:, i * 8 + g * GTILES + kk, :],
                                dst[:, g * GCOMPACT + kk * SCHUNK
                                       : g * GCOMPACT + (kk + 1) * SCHUNK])

            for _rep in range(reps):
                # ============ Phase A: load x, convert f16, BN1 stats ========
                # Last image loads in quarters so its final bn_stats (which
                # gate the AR1 collective) start a quarter-chunk earlier.
                for i in range(N_LOC):
                    ndma = 1
                    for h in range(2):
                        xin = work.tile([C, HW // 2], F32, tag="xin", bufs=3,
                                        name=f"xa{i}_{h}")
                        qsz = (HW // 2) // ndma
                        for q in range(ndma):
                            nc.sync.dma_start(
                                xin[:, q * qsz : (q + 1) * qsz],
                                x_d[i, :, h * (HW // 2) + q * qsz
                                      : h * (HW // 2) + (q + 1) * qsz])
                        xbsl = xb[:, i, h * (HW // 2) : (h + 1) * (HW // 2)]
                        nc.scalar.activation(xbsl, xin[:], AF.Copy)
                        for k in range(4):
                            # stats from the f32 data: the f16 copy shifts the
                            # BN1 mean enough (~1e-6) to flip near-threshold
                            # signs, which cascades through both binary convs
                            nc.vector.bn_stats(
                                stats1[:, i * 8 + h * 4 + k, :],
                                xin[:, k * SCHUNK : (k + 1) * SCHUNK])

                g1ar = reduce_stats(stats1, 0)
                mean1, rstd1 = _rstd_from_allreduced(nc, const, g1ar, "1")
                k1, c1b, tau1 = _affine_consts(
                    nc, const, pp, mean1, rstd1,
                    _col(pp, P_G1), _col(pp, P_NG1), P_B1, "1")

                def sign_to_pad(i, src_img, k, cb, tau):
                    """pad interior <- sign(k*src + cb) as +/-1 fp8.
                    DVE (2-op threshold) for some images, ACT for the rest.
                    High priority: the sign gates the PE for this image."""
                    pad = pads[i % N_PADS]
                    dst = pad[:, 1 : H + 1, 1 : W + 1]
                    with tc.high_priority(offset=60):
                        if DVE_SIGN and i in (1, 4, 6):
                            t01 = work.tile([C, HW], F16, tag="d1", bufs=3,
                                            name=f"t01_{i}")
                            nc.vector.tensor_scalar(t01[:], src_img, tau[:],
                                                    None, OP.is_ge)
                            nc.vector.tensor_scalar(
                                dst,
                                t01[:].rearrange("p (h w) -> p h w", h=H, w=W),
                                2.0, -1.0, OP.mult, OP.add)
                        else:
                            nc.scalar.activation(
                                dst,
                                src_img.rearrange("p (h w) -> p h w", h=H, w=W),
                                AF.Sign, bias=cb[:], scale=k[:],
                            )
                    return pad

                # ============ Phase B: b1 = sign(BN1(x)); conv1; stats2 ======
                # sign1 thresholds x near tau1 and errors cascade through two
                # binary convs, so it must read x at full f32 precision:
                # re-load x from HBM (prefetches during phase A / AR1).
                # sign halves split at row 30 so the first PSUM group (tiles
                # 0-3, which with backward zero-pairs reads pad rows <= 30)
                # depends only on the first half
                SIGN_SEG = [(0, 30), (30, 26)]
                for i in range(N_LOC):
                    pad = pads[i % N_PADS]
                    for r0, nr in SIGN_SEG:
                        ne = nr * W
                        with tc.high_priority(offset=60):
                            xin = work.tile([C, ne], F32, tag="xin",
                                            bufs=3, name=f"xs{i}_{r0}")
                            nc.sync.dma_start(
                                xin[:], x_d[i, :, r0 * W : (r0 + nr) * W])
                            dsth = pad[:, 1 + r0 : 1 + r0 + nr, 1 : W + 1]
                            if DVE_SIGN and i in (1, 4, 6):
                                t01 = work.tile([C, HW], F16, tag="d1",
                                                bufs=3, name=f"t01_{i}_{r0}")
                                t01h = t01[:, 0:ne]
                                nc.vector.tensor_scalar(t01h, xin[:], tau1[:],
                                                        None, OP.is_ge)
                                nc.vector.tensor_scalar(
                                    dsth,
                                    t01h.rearrange("p (h w) -> p h w",
                                                   h=nr, w=W),
                                    2.0, -1.0, OP.mult, OP.add)
                            else:
                                nc.scalar.activation(
                                    dsth,
                                    xin[:].rearrange("p (h w) -> p h w",
                                                     h=nr, w=W),
                                    AF.Sign, bias=c1b[:], scale=k1[:],
                                )
                    if dbg and i == 0:
                        nc.sync.dma_start(dbg_pad_d[:], pad[:])
                    conv(pad, w1s, c1f[:, i, :], stats2, i, a1, s1h)

                g2ar = reduce_stats(stats2, 1, r1, r1sq[:])
                mean2, rstd2 = _rstd_from_allreduced(nc, const, g2ar, "2")
                k2, c2b, tau2 = _affine_consts(
                    nc, const, pp, mean2, rstd2,
                    _col(pp, P_GR2), _col(pp, P_NGR2), P_B2, "2", rho=r1)

                # ============ Phase C: b2 = sign(BN2(p1)); conv2; stats3 =====
                for i in range(N_LOC):
                    pad = sign_to_pad(i, c1f[:, i, :], k2, c2b, tau2)
                    conv(pad, w2s, c2f[:, i, :], stats3, i, a2, s2h)

                g3ar = reduce_stats(stats3, 2, r2, r2sq[:])
                mean3, rstd3 = _rstd_from_allreduced(nc, const, g3ar, "3")
                k3, c3b, _tau3 = _affine_consts(
                    nc, const, pp, mean3, rstd3,
                    _col(pp, P_GR3), _col(pp, P_NGR3), P_B3, "3", rho=r2)

                if dbg:
                    nc.sync.dma_start(dbg_c1_d[:], c1f[:])
                    nc.sync.dma_start(dbg_c2_d[:], c2f[:])
                    dbgk = const.tile([C, 8], F32)
                    for j, t_ in enumerate(
                        [k1, c1b, tau1, k2, c2b, tau2, k3, c3b]
                    ):
                        nc.vector.tensor_copy(_col(dbgk, j), t_[:])
                    nc.sync.dma_start(dbg_k_d[:], dbgk[:])

                # ====== Phase D: y = PReLU(k3*p2 + x + c3b) ======
                for i in range(N_LOC):
                    d1 = work.tile([C, HW], F16, tag="d1", bufs=3)
                    nc.vector.tensor_scalar(d1[:], c2f[:, i, :], k3[:],
                                            None, OP.mult)
                    nc.vector.tensor_tensor(out=d1[:], in0=d1[:],
                                            in1=xb[:, i, :], op=OP.add)
                    if i >= N_LOC - 2:
                        # halve the trailing prelu+DMA chain of the last images
                        for h in range(2):
                            sl = slice(h * (HW // 2), (h + 1) * (HW // 2))
                            nc.scalar.activation(c1f[:, i, sl], d1[:, sl],
                                                 AF.Prelu, bias=c3b[:],
                                                 alpha=a3)
                            nc.sync.dma_start(out_d[i][:, sl], c1f[:, i, sl])
                    else:
                        nc.scalar.activation(c1f[:, i, :], d1[:], AF.Prelu,
                                             bias=c3b[:], alpha=a3)
                        nc.sync.dma_start(out_d[i], c1f[:, i, :])

    nc.compile()
    return nc


def _prep_host(x, bn1_g, bn1_b, w1, prelu1_a, bn2_g, bn2_b, w2, prelu2_a,
               bn3_g, bn3_b, prelu3_a):
    def wprep(w_flat):
        w = np.asarray(w_flat, np.float32).reshape(C, C, 3, 3)
        # lhsT layout [i, slot, o] = sign(w[o, i, dh, dw]); slot 9 zero-padded
        wT = np.sign(w).transpose(1, 2, 3, 0).reshape(C, 9, C)
        w10 = np.zeros((C, 10, C), np.float32)
        w10[:, :9, :] = wT
        s = np.mean(np.abs(w), axis=(1, 2, 3)).astype(np.float32)  # [C] per o
        s_hat = np.exp2(np.round(np.log2(s))).astype(np.float32)
        rho = (s / s_hat).astype(np.float32)
        return w10.astype(mybir.dt.np(FP8)), s_hat, rho

    w1t, s1h_, r1_ = wprep(w1)
    w2t, s2h_, r2_ = wprep(w2)

    pp = np.zeros((C, NP), np.float32)
    pp[:, P_S1H] = s1h_
    pp[:, P_R1] = r1_
    pp[:, P_S2H] = s2h_
    pp[:, P_R2] = r2_
    g1_ = np.asarray(bn1_g, np.float32)
    g2_ = np.asarray(bn2_g, np.float32)
    g3_ = np.asarray(bn3_g, np.float32)
    pp[:, P_NG1] = -g1_
    pp[:, P_GR2] = g2_ * r1_
    pp[:, P_NGR2] = -(g2_ * r1_)
    pp[:, P_GR3] = g3_ * r2_
    pp[:, P_NGR3] = -(g3_ * r2_)
    pp[:, P_G1] = np.asarray(bn1_g, np.float32)
    pp[:, P_B1] = np.asarray(bn1_b, np.float32)
    pp[:, P_G2] = np.asarray(bn2_g, np.float32)
    pp[:, P_B2] = np.asarray(bn2_b, np.float32)
    pp[:, P_G3] = np.asarray(bn3_g, np.float32)
    pp[:, P_B3] = np.asarray(bn3_b, np.float32)
    pp[:, P_A1] = np.float32(prelu1_a)
    pp[:, P_A2] = np.float32(prelu2_a)
    pp[:, P_A3] = np.float32(prelu3_a)

    x = np.ascontiguousarray(np.asarray(x, np.float32).reshape(64, C, HW))
    in_maps = []
    for r in range(N_CORES):
        in_maps.append({
            "x": x[r * N_LOC : (r + 1) * N_LOC],
            "w1t": w1t,
            "w2t": w2t,
            "pp": pp,
        })
    return in_maps


_NC_CACHE = None


def _get_nc():
    global _NC_CACHE
    if _NC_CACHE is None:
        _NC_CACHE = build_nc()
    return _NC_CACHE


def run(in_maps, **kwargs):
    nc = _get_nc()
    return run_bass_kernel_spmd(nc, in_maps, core_ids=list(range(N_CORES)), **kwargs)


def kernel(**inputs):
    in_maps = _prep_host(**inputs)
    last_err = None
    for attempt in range(3):
        try:
            res = run(in_maps)
            break
        except Exception as e:  # transient NRT device errors happen; retry
            last_err = e
            import time as _time
            _time.sleep(2.0)
    else:
        raise last_err
    out = np.concatenate(
        [np.asarray(r["out"]).astype(np.float32).reshape(N_LOC, C, H, W)
         for r in res.results], axis=0
    )
    return out


if __name__ == "__main__":
    rng = np.random.default_rng(0)
    x = rng.standard_normal((64, C, H, W), dtype=np.float32)
    w1 = ((rng.random((C * C * 9, 1), dtype=np.float32) - 0.5) * 0.002)
    w2 = ((rng.random((C * C * 9, 1), dtype=np.float32) - 0.5) * 0.002)
    ones = np.ones(C, np.float32)
    zeros = np.zeros(C, np.float32)
    y = kernel(x=x, bn1_g=ones, bn1_b=zeros, w1=w1, prelu1_a=np.float32(0.25),
               bn2_g=ones, bn2_b=zeros, w2=w2, prelu2_a=np.float32(0.25),
               bn3_g=ones, bn3_b=zeros, prelu3_a=np.float32(0.25))
    print("out", y.shape, y.dtype, float(np.abs(y).mean()))
